# revision 1
# baseline (speedup 1.0000x reference)
"""Self-contained Trainium2 Bass kernel for multi-head causal attention with RoPE.

Problem: B=2, S=2048, D=2048, H=16 heads (HD=128), fp32 reference:
    q = rope(x @ wq.T), k = rope(x @ wk.T), v = x @ wv.T
    out = softmax(q k^T / sqrt(HD) + causal_mask) @ v @ wo.T

Sharding (8 cores): core c = (b, g) with b = c // 4 (batch), g = c % 4
(head-group of 4 heads).  Each core computes its head-group's attention for
its batch and a partial output projection (columns 512g:512g+512 of the
attention output times the matching wo rows).  The host sums the 4 partial
[D, S] tensors per batch and transposes back to [S, D].

On-chip layout is "transposed": Q^T/K^T are kept as [head_dim, seq] so the
QK^T matmul needs no transposes, scores come out as scoresT[k, q], the
softmax denominator is a ones-vector matmul on the PE, and probsT feeds the
PV matmul directly (lhsT = V[sk, e]).  RoPE's even/odd pair mixing becomes a
contiguous half-partition mix because the head dims of wq/wk are permuted
host-side (evens first, odds second) — a permutation that cancels inside
q·k.  Causal masking: strictly-above-diagonal 128x512 score tiles are
skipped entirely; the 4 diagonal tile flavours get an additive -60000 mask.
"""

import math
import os

import numpy as np
import ml_dtypes

import concourse.bass as bass
import concourse.bacc as bacc
import concourse.mybir as mybir
from concourse.tile import TileContext
from concourse.bass_utils import run_bass_kernel_spmd
from contextlib import ExitStack

B, S, D, H = 2, 2048, 2048, 16
HD = 128          # head dim
HPG = 4           # heads per core (group)
EG = HPG * HD     # 512 head dims per core
NCORES = 8
NSTRIP = 4        # q strips per sequence
STRIP = S // NSTRIP   # 512
SKT = 128         # k tile (partition dim of scoresT)
NDT = D // 128    # 16 contraction tiles for projections
SCALE = 1.0 / math.sqrt(HD)

BF16 = mybir.dt.bfloat16
F32 = mybir.dt.float32

LAST_EXEC_NS = None
LAST_RESULTS = None


def _build_program():
    nc = bacc.Bacc("TRN2", target_bir_lowering=False, debug=False,
                   num_devices=NCORES)
    xT_d = nc.dram_tensor("xT", [D, S], BF16, kind="ExternalInput").ap()
    wqT_d = nc.dram_tensor("wqT", [D, EG], BF16, kind="ExternalInput").ap()
    wkT_d = nc.dram_tensor("wkT", [D, EG], BF16, kind="ExternalInput").ap()
    wvT_d = nc.dram_tensor("wvT", [D, EG], BF16, kind="ExternalInput").ap()
    woT_d = nc.dram_tensor("woT", [EG, D], BF16, kind="ExternalInput").ap()
    cs_d = nc.dram_tensor("cs", [HD, S], BF16, kind="ExternalInput").ap()
    sn_d = nc.dram_tensor("sn", [HD, S], BF16, kind="ExternalInput").ap()
    mk_d = nc.dram_tensor("mk", [SKT, 4, STRIP], F32, kind="ExternalInput").ap()
    jt_d = nc.dram_tensor("jt", [HD, HD], BF16, kind="ExternalInput").ap()
    outT_d = nc.dram_tensor("outT", [D, S], F32, kind="ExternalOutput").ap()

    EXP = mybir.ActivationFunctionType.Exp

    with TileContext(nc) as tc, ExitStack() as ctx:
        wpool = ctx.enter_context(tc.tile_pool(name="wpool", bufs=1))
        kv = ctx.enter_context(tc.tile_pool(name="kv", bufs=1))
        xs = ctx.enter_context(tc.tile_pool(name="xs", bufs=1))
        qs = ctx.enter_context(tc.tile_pool(name="qs", bufs=2))
        rp = ctx.enter_context(tc.tile_pool(name="rp", bufs=3))
        ep = ctx.enter_context(tc.tile_pool(name="ep", bufs=6))
        ot = ctx.enter_context(tc.tile_pool(name="ot", bufs=2))
        po = ctx.enter_context(tc.tile_pool(name="po", bufs=3))
        nrm = ctx.enter_context(tc.tile_pool(name="nrm", bufs=2))
        mmps = ctx.enter_context(tc.tile_pool(name="mmps", bufs=3, space="PSUM"))
        pvps = ctx.enter_context(tc.tile_pool(name="pvps", bufs=2, space="PSUM"))
        smps = ctx.enter_context(tc.tile_pool(name="smps", bufs=1, space="PSUM"))
        wops = ctx.enter_context(tc.tile_pool(name="wops", bufs=2, space="PSUM"))

        # chunked weight preloads: first projection matmuls only gate on the
        # first 4 d-tiles of wq, not the whole 6MB weight preload
        wq_sb = wpool.tile([128, NDT, EG], BF16)
        wk_sb = wpool.tile([128, NDT, EG], BF16)
        wv_sb = wpool.tile([128, NDT, EG], BF16)
        for w_sb, w_d in ((wq_sb, wqT_d), (wk_sb, wkT_d), (wv_sb, wvT_d)):
            wr = w_d.rearrange("(t p) e -> p t e", p=128)
            for c0 in range(0, NDT, 4):
                nc.sync.dma_start(out=w_sb[:, c0:c0 + 4, :],
                                  in_=wr[:, c0:c0 + 4, :])
        wo_sb = wpool.tile([128, HPG, D], BF16)
        nc.sync.dma_start(out=wo_sb, in_=woT_d.rearrange("(t p) n -> p t n", p=128))
        cs_sb = wpool.tile([128, S], BF16)
        nc.sync.dma_start(out=cs_sb, in_=cs_d)
        sn_sb = wpool.tile([128, S], BF16)
        nc.sync.dma_start(out=sn_sb, in_=sn_d)
        mk_sb = wpool.tile([128, 4, STRIP], F32)
        nc.sync.dma_start(out=mk_sb, in_=mk_d)
        ones_sb = wpool.tile([128, 1], BF16)
        nc.vector.memset(ones_sb, 1.0)
        onesc_sb = wpool.tile([1, 128], F32)
        nc.vector.memset(onesc_sb, 1.0)
        jt_sb = wpool.tile([HD, HD], BF16)
        nc.sync.dma_start(out=jt_sb, in_=jt_d)

        KT_sb = kv.tile([128, HPG, S], BF16)     # [e, h, sk] rope'd K^T
        V_sb = kv.tile([128, S // 128, EG], BF16)  # [sk, sk_tile, e]

        for j in range(NSTRIP):
            s0 = j * STRIP
            xt = xs.tile([128, NDT, STRIP], BF16, tag="xt")
            nc.sync.dma_start(
                out=xt,
                in_=xT_d[:, s0:s0 + STRIP].rearrange("(t p) s -> p t s", p=128))
            qt = qs.tile([128, HPG, STRIP], BF16, tag="qt")

            # --- projections + RoPE for this strip ---
            for h in range(HPG):
                e0 = h * HD
                q_ps = mmps.tile([128, STRIP], F32, tag="mm")
                for dt in range(NDT):
                    nc.tensor.matmul(q_ps, lhsT=wq_sb[:, dt, e0:e0 + HD],
                                     rhs=xt[:, dt, :],
                                     start=(dt == 0), stop=(dt == NDT - 1))
                q_sb = rp.tile([128, STRIP], BF16, tag="qsb")
                nc.scalar.copy(q_sb, q_ps)
                jq_ps = mmps.tile([128, STRIP], F32, tag="mm")
                nc.tensor.matmul(jq_ps, lhsT=jt_sb, rhs=q_sb,
                                 start=True, stop=True)
                t1 = rp.tile([128, STRIP], F32, tag="ra")
                nc.vector.tensor_mul(t1, q_sb, cs_sb[:, s0:s0 + STRIP])
                t2 = rp.tile([128, STRIP], F32, tag="rb")
                nc.vector.tensor_mul(t2, jq_ps, sn_sb[:, s0:s0 + STRIP])
                nc.vector.tensor_add(qt[:, h, :], t1, t2)

                k_ps = mmps.tile([128, STRIP], F32, tag="mm")
                for dt in range(NDT):
                    nc.tensor.matmul(k_ps, lhsT=wk_sb[:, dt, e0:e0 + HD],
                                     rhs=xt[:, dt, :],
                                     start=(dt == 0), stop=(dt == NDT - 1))
                k_sb = rp.tile([128, STRIP], BF16, tag="qsb")
                nc.scalar.copy(k_sb, k_ps)
                jk_ps = mmps.tile([128, STRIP], F32, tag="mm")
                nc.tensor.matmul(jk_ps, lhsT=jt_sb, rhs=k_sb,
                                 start=True, stop=True)
                t3 = rp.tile([128, STRIP], F32, tag="ra")
                nc.vector.tensor_mul(t3, k_sb, cs_sb[:, s0:s0 + STRIP])
                t4 = rp.tile([128, STRIP], F32, tag="rb")
                nc.vector.tensor_mul(t4, jk_ps, sn_sb[:, s0:s0 + STRIP])
                nc.vector.tensor_add(KT_sb[:, h, s0:s0 + STRIP], t3, t4)

            for st in range(4):
                v_ps = mmps.tile([128, EG], F32, tag="mm")
                for dt in range(NDT):
                    nc.tensor.matmul(v_ps, lhsT=xt[:, dt, st * 128:(st + 1) * 128],
                                     rhs=wv_sb[:, dt, :],
                                     start=(dt == 0), stop=(dt == NDT - 1))
                nc.scalar.copy(V_sb[:, j * 4 + st, :], v_ps)

            # --- attention for this strip ---
            otile = ot.tile([128, HPG, STRIP], BF16, tag="ot")
            nsk = 4 * j + 4
            for h in range(HPG):
                e0 = h * HD
                pv_ps = pvps.tile([128, STRIP], F32, tag="pv")
                sm_ps = smps.tile([1, STRIP], F32, tag="sm")
                for skt in range(nsk):
                    sc_ps = mmps.tile([128, STRIP], F32, tag="mm")
                    nc.tensor.matmul(sc_ps,
                                     lhsT=KT_sb[:, h, skt * 128:(skt + 1) * 128],
                                     rhs=qt[:, h, :], start=True, stop=True)
                    if skt >= 4 * j:
                        nc.vector.tensor_add(sc_ps, sc_ps, mk_sb[:, skt - 4 * j, :])
                    ex = ep.tile([128, STRIP], BF16, tag="ex")
                    nc.scalar.activation(ex, sc_ps, EXP, scale=SCALE)
                    nc.tensor.matmul(pv_ps, lhsT=V_sb[:, skt, e0:e0 + HD],
                                     rhs=ex, start=(skt == 0), stop=(skt == nsk - 1))
                    nc.tensor.matmul(sm_ps, lhsT=ones_sb, rhs=ex,
                                     start=(skt == 0), stop=(skt == nsk - 1))
                rc = nrm.tile([1, STRIP], F32, tag="rc")
                nc.vector.reciprocal(rc, sm_ps)
                # broadcast recip across partitions: ones[1,128].T @ rc[1,512]
                bc_ps = mmps.tile([128, STRIP], F32, tag="mm")
                nc.tensor.matmul(bc_ps, lhsT=onesc_sb, rhs=rc,
                                 start=True, stop=True)
                bc = nrm.tile([128, STRIP], F32, tag="bc")
                nc.scalar.copy(bc, bc_ps)
                nc.vector.tensor_mul(otile[:, h, :], pv_ps, bc)

            # --- partial output projection for this strip ---
            for nt in range(NDT):
                pr = wops.tile([128, STRIP], F32, tag="pr")
                for h in range(HPG):
                    nc.tensor.matmul(pr, lhsT=wo_sb[:, h, nt * 128:(nt + 1) * 128],
                                     rhs=otile[:, h, :],
                                     start=(h == 0), stop=(h == HPG - 1))
                pr_sb = po.tile([128, STRIP], F32, tag="po")
                nc.vector.tensor_copy(pr_sb, pr)
                nc.sync.dma_start(
                    out=outT_d[nt * 128:(nt + 1) * 128, s0:s0 + STRIP], in_=pr_sb)

    return nc


_PERM = np.concatenate([np.arange(0, HD, 2), np.arange(1, HD, 2)])


def _host_prep(x, wq, wk, wv, wo, freqs_cos, freqs_sin, mask):
    bf16 = ml_dtypes.bfloat16
    x = np.asarray(x, np.float32)
    wq = np.asarray(wq, np.float32)
    wk = np.asarray(wk, np.float32)
    wv = np.asarray(wv, np.float32)
    wo = np.asarray(wo, np.float32)
    cos = np.asarray(freqs_cos, np.float32)   # [S, HD/2]
    sin = np.asarray(freqs_sin, np.float32)
    mask = np.asarray(mask, np.float32)

    cosH = cos.T                               # [64, S]
    sinH = sin.T
    cs = np.vstack([cosH, cosH]).astype(bf16)  # [128, S]
    sn = np.vstack([sinH, sinH]).astype(bf16)

    # additive causal mask for the 4 diagonal-tile flavours: mk[k, d, q]
    mk = np.empty((SKT, 4, STRIP), np.float32)
    for d_ in range(4):
        sub = mask[0:STRIP, d_ * SKT:(d_ + 1) * SKT]   # [q, k]
        mk[:, d_, :] = np.where(np.isfinite(sub), 0.0, -60000.0).T

    perm_g = np.concatenate([h * HD + _PERM for h in range(HPG)])

    # lhsT of the rope pair-mix matmul: (J q) rows 0:64 = -q[64:128],
    # rows 64:128 = +q[0:64]; jt = J.T
    jt = np.zeros((HD, HD), np.float32)
    jt[np.arange(64), np.arange(64) + 64] = 1.0
    jt[np.arange(64) + 64, np.arange(64)] = -1.0
    jt = jt.astype(bf16)

    in_maps = []
    for c in range(NCORES):
        b, g = c // HPG, c % HPG
        rows = slice(g * EG, (g + 1) * EG)
        wq_g = wq[rows][perm_g]                # [EG, D], head dims permuted
        wk_g = wk[rows][perm_g]
        wv_g = wv[rows]
        in_maps.append({
            "xT": np.ascontiguousarray(x[b].T).astype(bf16),
            "wqT": np.ascontiguousarray(wq_g.T).astype(bf16),
            "wkT": np.ascontiguousarray(wk_g.T).astype(bf16),
            "wvT": np.ascontiguousarray(wv_g.T).astype(bf16),
            "woT": np.ascontiguousarray(wo[:, rows].T).astype(bf16),
            "cs": cs, "sn": sn, "mk": mk, "jt": jt,
        })
    return in_maps


def kernel(x, wq, wk, wv, wo, freqs_cos, freqs_sin, mask, start_pos):
    global LAST_EXEC_NS, LAST_RESULTS
    in_maps = _host_prep(x, wq, wk, wv, wo, freqs_cos, freqs_sin, mask)
    nc = _build_program()
    nc.finalize()
    res = run_bass_kernel_spmd(nc, in_maps, core_ids=list(range(NCORES)),
                               trace=False)
    LAST_EXEC_NS = res.exec_time_ns
    LAST_RESULTS = res
    out = np.empty((B, S, D), np.float32)
    for b in range(B):
        acc = np.zeros((D, S), np.float32)
        for g in range(HPG):
            acc += res.results[b * HPG + g]["outT"]
        out[b] = acc.T
    return out



# revision 21
# speedup vs baseline: 1.4801x; 1.4801x over previous
"""Self-contained Trainium2 Bass kernel for multi-head causal attention with RoPE.

Problem: B=2, S=2048, D=2048, H=16 heads (HD=128), fp32 reference:
    q = rope(x @ wq.T), k = rope(x @ wk.T), v = x @ wv.T
    out = softmax(q k^T / sqrt(HD) + causal_mask) @ v @ wo.T

Sharding (8 cores): core c = (b, g) with b = c // 4 (batch), g = c % 4
(head-group of 4 heads).  Each core computes its head-group's attention for
its batch and a partial output projection (columns 512g:512g+512 of the
attention output times the matching wo rows).  The host sums the 4 partial
[D, S] tensors per batch and transposes back to [S, D].

On-chip layout is "transposed": Q^T/K^T are kept as [head_dim, seq] so the
QK^T matmul needs no transposes, scores come out as scoresT[k, q], and
probsT feeds the PV matmul directly (lhsT = V[sk, e]).  RoPE's even/odd pair
mixing becomes a half-partition mix via a PE matmul against a signed
permutation J (head dims of wq/wk are permuted host-side, which cancels in
q.k).  Causal masking: strictly-above-diagonal 128x512 score tiles are
skipped; the 4 diagonal-tile flavours get a multiplicative binary bf16 mask
applied after exp.  Softmax denominators: exp tiles are summed in quads on
DVE (bf16 tree) and reduced over partitions with a ones-vector matmul; the
reciprocal is broadcast across partitions on GpSimd (attn ucode library).
"""

import math

import numpy as np
import ml_dtypes

import concourse.bass as bass
import concourse.bacc as bacc
import concourse.mybir as mybir
from concourse.tile import TileContext
from concourse.bass_utils import run_bass_kernel_spmd
from concourse import library_config
from concourse.bass_isa import ReduceOp
from contextlib import ExitStack

B, S, D, H = 2, 2048, 2048, 16
HD = 128          # head dim
HPG = 4           # heads per core (group)
EG = HPG * HD     # 512 head dims per core
NCORES = 8
NSTRIP = 4        # q strips per sequence
STRIP = S // NSTRIP   # 512
SKT = 128         # k tile (partition dim of scoresT)
NDT = D // 128    # 16 contraction tiles for projections
SCALE = 1.0 / math.sqrt(HD)

BF16 = mybir.dt.bfloat16
F32 = mybir.dt.float32

LAST_EXEC_NS = None
LAST_RESULTS = None


def _build_program():
    nc = bacc.Bacc("TRN2", target_bir_lowering=False, debug=False,
                   num_devices=NCORES)
    xT_d = nc.dram_tensor("xT", [D, S], BF16, kind="ExternalInput").ap()
    wqT_d = nc.dram_tensor("wqT", [D, EG], BF16, kind="ExternalInput").ap()
    wkT_d = nc.dram_tensor("wkT", [D, EG], BF16, kind="ExternalInput").ap()
    wvT_d = nc.dram_tensor("wvT", [D, EG], BF16, kind="ExternalInput").ap()
    woT_d = nc.dram_tensor("woT", [EG, D], BF16, kind="ExternalInput").ap()
    cs_d = nc.dram_tensor("cs", [HD, S], BF16, kind="ExternalInput").ap()
    sn_d = nc.dram_tensor("sn", [HD, S], BF16, kind="ExternalInput").ap()
    mk_d = nc.dram_tensor("mk", [SKT, 4, STRIP], BF16, kind="ExternalInput").ap()
    jt_d = nc.dram_tensor("jt", [HD, HD], BF16, kind="ExternalInput").ap()
    outT_d = nc.dram_tensor("outT", [D, S], BF16, kind="ExternalOutput").ap()

    EXP = mybir.ActivationFunctionType.Exp

    with TileContext(nc) as tc, ExitStack() as ctx:
        nc.gpsimd.load_library(library_config.attn)

        wpool = ctx.enter_context(tc.tile_pool(name="wpool", bufs=1))
        kv = ctx.enter_context(tc.tile_pool(name="kv", bufs=1))
        xs = ctx.enter_context(tc.tile_pool(name="xs", bufs=2))
        qs = ctx.enter_context(tc.tile_pool(name="qs", bufs=2))
        rp = ctx.enter_context(tc.tile_pool(name="rp", bufs=3))
        ep = ctx.enter_context(tc.tile_pool(name="ep", bufs=5))
        ot = ctx.enter_context(tc.tile_pool(name="ot", bufs=2))
        po = ctx.enter_context(tc.tile_pool(name="po", bufs=5))
        nrm = ctx.enter_context(tc.tile_pool(name="nrm", bufs=2))
        rcbp = ctx.enter_context(tc.tile_pool(name="rcbp", bufs=2))
        scps = ctx.enter_context(tc.tile_pool(name="scps", bufs=2, space="PSUM"))
        prps = ctx.enter_context(tc.tile_pool(name="prps", bufs=2, space="PSUM"))
        pvps = ctx.enter_context(tc.tile_pool(name="pvps", bufs=2, space="PSUM"))
        wops = ctx.enter_context(tc.tile_pool(name="wops", bufs=2, space="PSUM"))

        # persistent SBUF tensors
        wq_sb = wpool.tile([128, NDT, EG], BF16)
        wk_sb = wpool.tile([128, NDT, EG], BF16)
        wv_sb = wpool.tile([128, NDT, EG], BF16)
        wo_sb = wpool.tile([128, HPG, D], BF16)
        cs_sb = wpool.tile([128, S], BF16)
        sn_sb = wpool.tile([128, S], BF16)
        mk_sb = wpool.tile([128, 4, STRIP], BF16)
        jt_sb = wpool.tile([HD, HD], BF16)
        KT_sb = kv.tile([128, HPG, S], BF16)       # [e, h, sk] rope'd K^T
        V_sb = kv.tile([128, S // 128, EG], BF16)  # [sk, sk_tile, e]

        xt0 = xs.tile([128, NDT, STRIP], BF16, tag="xt")
        x0r = xT_d[:, 0:STRIP].rearrange("(t p) s -> p t s", p=128)

        # --- DMA issue order tuned for the critical path: the first q
        # projection needs wq chunk 0 + the first x d-tiles.
        wqr = wqT_d.rearrange("(t p) e -> p t e", p=128)
        wkr = wkT_d.rearrange("(t p) e -> p t e", p=128)
        wvr = wvT_d.rearrange("(t p) e -> p t e", p=128)
        for c0 in range(0, NDT, 4):
            nc.sync.dma_start(out=wq_sb[:, c0:c0 + 4, :], in_=wqr[:, c0:c0 + 4, :])
            nc.sync.dma_start(out=xt0[:, c0:c0 + 4, :], in_=x0r[:, c0:c0 + 4, :])
        xt1 = xs.tile([128, NDT, STRIP], BF16, tag="xt")
        nc.sync.dma_start(
            out=xt1,
            in_=xT_d[:, STRIP:2 * STRIP].rearrange("(t p) s -> p t s", p=128))
        nc.sync.dma_start(out=cs_sb, in_=cs_d)
        nc.sync.dma_start(out=sn_sb, in_=sn_d)
        nc.sync.dma_start(out=jt_sb, in_=jt_d)
        for c0 in range(0, NDT, 4):
            nc.sync.dma_start(out=wk_sb[:, c0:c0 + 4, :], in_=wkr[:, c0:c0 + 4, :])
        nc.sync.dma_start(out=mk_sb, in_=mk_d)
        for c0 in range(0, NDT, 4):
            nc.sync.dma_start(out=wv_sb[:, c0:c0 + 4, :], in_=wvr[:, c0:c0 + 4, :])
        nc.sync.dma_start(out=wo_sb, in_=woT_d.rearrange("(t p) n -> p t n", p=128))

        for j in range(NSTRIP):
            s0 = j * STRIP
            if j == 0:
                xt = xt0
            elif j == 1:
                xt = xt1
            else:
                xt = xs.tile([128, NDT, STRIP], BF16, tag="xt")
                nc.sync.dma_start(
                    out=xt,
                    in_=xT_d[:, s0:s0 + STRIP].rearrange("(t p) s -> p t s", p=128))
            qt = qs.tile([128, HPG, STRIP], BF16, tag="qt")

            # --- projections + RoPE for this strip ---
            if j == 0:
                # strip 0: chunk-outer over the 4-dt DMA chunks so the first
                # matmuls start as soon as (wq chunk 0, x chunk 0) land; the
                # 4 concurrent head accumulators borrow the idle sc psum bufs
                qp0 = prps.tile([128, STRIP], F32, tag="mm")
                qp1 = prps.tile([128, STRIP], F32, tag="mm")
                qp2 = scps.tile([128, STRIP], F32, tag="sc")
                qp3 = scps.tile([128, STRIP], F32, tag="sc")
                qps0 = [qp0, qp1, qp2, qp3]
                for c0 in range(0, NDT, 4):
                    for h in range(HPG):
                        e0 = h * HD
                        for dt in range(c0, c0 + 4):
                            nc.tensor.matmul(qps0[h],
                                             lhsT=wq_sb[:, dt, e0:e0 + HD],
                                             rhs=xt[:, dt, :],
                                             start=(dt == 0),
                                             stop=(dt == NDT - 1))
            for h in range(HPG):
                e0 = h * HD
                if j == 0:
                    q_ps = qps0[h]
                else:
                    q_ps = prps.tile([128, STRIP], F32, tag="mm")
                    for dt in range(NDT):
                        nc.tensor.matmul(q_ps, lhsT=wq_sb[:, dt, e0:e0 + HD],
                                         rhs=xt[:, dt, :],
                                         start=(dt == 0), stop=(dt == NDT - 1))
                q_sb = rp.tile([128, STRIP], BF16, tag="qsb")
                nc.scalar.copy(q_sb, q_ps)
                if j == 0:
                    jq_ps = pvps.tile([128, STRIP], F32, tag="pv")
                else:
                    jq_ps = prps.tile([128, STRIP], F32, tag="mm")
                nc.tensor.matmul(jq_ps, lhsT=jt_sb, rhs=q_sb,
                                 start=True, stop=True)
                t1 = rp.tile([128, STRIP], BF16, tag="ra")
                nc.vector.tensor_mul(t1, q_sb, cs_sb[:, s0:s0 + STRIP])
                t2 = rp.tile([128, STRIP], BF16, tag="rb")
                nc.vector.tensor_mul(t2, jq_ps, sn_sb[:, s0:s0 + STRIP])
                nc.vector.tensor_add(qt[:, h, :], t1, t2)

                k_ps = prps.tile([128, STRIP], F32, tag="mm")
                for dt in range(NDT):
                    nc.tensor.matmul(k_ps, lhsT=wk_sb[:, dt, e0:e0 + HD],
                                     rhs=xt[:, dt, :],
                                     start=(dt == 0), stop=(dt == NDT - 1))
                k_sb = rp.tile([128, STRIP], BF16, tag="qsb")
                nc.scalar.copy(k_sb, k_ps)
                jk_ps = prps.tile([128, STRIP], F32, tag="mm")
                nc.tensor.matmul(jk_ps, lhsT=jt_sb, rhs=k_sb,
                                 start=True, stop=True)
                t3 = rp.tile([128, STRIP], BF16, tag="ra")
                nc.vector.tensor_mul(t3, k_sb, cs_sb[:, s0:s0 + STRIP])
                t4 = rp.tile([128, STRIP], BF16, tag="rb")
                nc.vector.tensor_mul(t4, jk_ps, sn_sb[:, s0:s0 + STRIP])
                nc.vector.tensor_add(KT_sb[:, h, s0:s0 + STRIP], t3, t4)

            for st in range(4):
                v_ps = prps.tile([128, EG], F32, tag="mm")
                for dt in range(NDT):
                    nc.tensor.matmul(v_ps, lhsT=xt[:, dt, st * 128:(st + 1) * 128],
                                     rhs=wv_sb[:, dt, :],
                                     start=(dt == 0), stop=(dt == NDT - 1))
                nc.scalar.copy(V_sb[:, j * 4 + st, :], v_ps)

            # --- attention for this strip ---
            otile = ot.tile([128, HPG, STRIP], BF16, tag="ot")
            nsk = 4 * j + 4
            for h in range(HPG):
                e0 = h * HD
                pv_ps = pvps.tile([128, STRIP], F32, tag="pv")
                acc = None   # bf16 running elementwise sum of masked exps;
                #              the 128-partition reduction happens in f32 on
                #              Pool, so bf16 chain rounding averages out
                for skt in range(nsk):
                    d = skt - 4 * j   # >= 0 on the diagonal block
                    w = STRIP - 128 * d if d > 0 else STRIP
                    sc_ps = scps.tile([128, STRIP], F32, tag="sc")
                    nc.tensor.matmul(sc_ps[:, 0:w],
                                     lhsT=KT_sb[:, h, skt * 128:(skt + 1) * 128],
                                     rhs=qt[:, h, STRIP - w:STRIP],
                                     start=True, stop=True)
                    ex = ep.tile([128, STRIP], BF16, tag="ex")
                    nc.scalar.activation(ex[:, 0:w], sc_ps[:, 0:w], EXP,
                                         scale=SCALE)
                    if d >= 0:
                        exm = ep.tile([128, STRIP], BF16, tag="exm")
                        nc.vector.tensor_mul(exm[:, 0:w], ex[:, 0:w],
                                             mk_sb[:, d, STRIP - w:STRIP])
                    else:
                        exm = ex
                    nc.tensor.matmul(pv_ps[:, STRIP - w:STRIP],
                                     lhsT=V_sb[:, skt, e0:e0 + HD],
                                     rhs=exm[:, 0:w], start=(skt == 0),
                                     stop=(skt == nsk - 1))
                    if skt == 0:
                        acc = exm      # full width; later adds are in-place
                    else:
                        nc.vector.tensor_add(acc[:, STRIP - w:STRIP],
                                             acc[:, STRIP - w:STRIP],
                                             exm[:, 0:w])
                den = nrm.tile([128, STRIP], F32, tag="den")
                nc.gpsimd.partition_all_reduce(den, acc, 128, ReduceOp.add)
                rcb = rcbp.tile([128, STRIP], F32, tag="rcb")
                nc.vector.reciprocal(rcb, den)
                nc.vector.tensor_mul(otile[:, h, :], pv_ps, rcb)

            # --- partial output projection for this strip ---
            for nt in range(NDT):
                if j == NSTRIP - 1:
                    # last strip: no next-strip projections exist to fill PE,
                    # so spread wo groups over the now-idle proj/score psum
                    # banks; heads 0-2 accumulate while head 3 finishes
                    if nt % 2 == 0:
                        pr = wops.tile([128, STRIP], F32, tag="pr")
                    elif nt % 4 == 1:
                        pr = prps.tile([128, STRIP], F32, tag="mm")
                    else:
                        pr = scps.tile([128, STRIP], F32, tag="sc")
                else:
                    pr = wops.tile([128, STRIP], F32, tag="pr")
                for h in range(HPG):
                    nc.tensor.matmul(pr, lhsT=wo_sb[:, h, nt * 128:(nt + 1) * 128],
                                     rhs=otile[:, h, :],
                                     start=(h == 0), stop=(h == HPG - 1))
                pr_sb = po.tile([128, STRIP], BF16, tag="po")
                if nt % 2 == 0:
                    nc.vector.tensor_copy(pr_sb, pr)
                else:
                    nc.scalar.copy(pr_sb, pr)
                nc.sync.dma_start(
                    out=outT_d[nt * 128:(nt + 1) * 128, s0:s0 + STRIP], in_=pr_sb)

    return nc


_PERM = np.concatenate([np.arange(0, HD, 2), np.arange(1, HD, 2)])


def _host_prep(x, wq, wk, wv, wo, freqs_cos, freqs_sin, mask):
    bf16 = ml_dtypes.bfloat16
    x = np.asarray(x, np.float32)
    wq = np.asarray(wq, np.float32)
    wk = np.asarray(wk, np.float32)
    wv = np.asarray(wv, np.float32)
    wo = np.asarray(wo, np.float32)
    cos = np.asarray(freqs_cos, np.float32)   # [S, HD/2]
    sin = np.asarray(freqs_sin, np.float32)
    mask = np.asarray(mask, np.float32)

    cosH = cos.T                               # [64, S]
    sinH = sin.T
    cs = np.vstack([cosH, cosH]).astype(bf16)  # [128, S]
    sn = np.vstack([sinH, sinH]).astype(bf16)

    # multiplicative binary causal mask for the 4 diagonal-tile flavours:
    # mk[k, d, q] = 1 where allowed, 0 where masked
    mk = np.empty((SKT, 4, STRIP), np.float32)
    for d_ in range(4):
        sub = mask[0:STRIP, d_ * SKT:(d_ + 1) * SKT]   # [q, k]
        mk[:, d_, :] = np.where(np.isfinite(sub), 1.0, 0.0).T
    mk = mk.astype(bf16)

    perm_g = np.concatenate([h * HD + _PERM for h in range(HPG)])

    # lhsT of the rope pair-mix matmul: (J q) rows 0:64 = -q[64:128],
    # rows 64:128 = +q[0:64]; jt = J.T
    jt = np.zeros((HD, HD), np.float32)
    jt[np.arange(64), np.arange(64) + 64] = 1.0
    jt[np.arange(64) + 64, np.arange(64)] = -1.0
    jt = jt.astype(bf16)

    in_maps = []
    for c in range(NCORES):
        b, g = c // HPG, c % HPG
        rows = slice(g * EG, (g + 1) * EG)
        wq_g = wq[rows][perm_g]                # [EG, D], head dims permuted
        wk_g = wk[rows][perm_g]
        wv_g = wv[rows]
        in_maps.append({
            "xT": np.ascontiguousarray(x[b].T).astype(bf16),
            "wqT": np.ascontiguousarray(wq_g.T).astype(bf16),
            "wkT": np.ascontiguousarray(wk_g.T).astype(bf16),
            "wvT": np.ascontiguousarray(wv_g.T).astype(bf16),
            "woT": np.ascontiguousarray(wo[:, rows].T).astype(bf16),
            "cs": cs, "sn": sn, "mk": mk, "jt": jt,
        })
    return in_maps


def kernel(x, wq, wk, wv, wo, freqs_cos, freqs_sin, mask, start_pos):
    global LAST_EXEC_NS, LAST_RESULTS
    in_maps = _host_prep(x, wq, wk, wv, wo, freqs_cos, freqs_sin, mask)
    nc = _build_program()
    nc.finalize()
    res = run_bass_kernel_spmd(nc, in_maps, core_ids=list(range(NCORES)),
                               trace=False)
    LAST_EXEC_NS = res.exec_time_ns
    LAST_RESULTS = res
    out = np.empty((B, S, D), np.float32)
    for b in range(B):
        acc = np.zeros((D, S), np.float32)
        for g in range(HPG):
            acc += res.results[b * HPG + g]["outT"].astype(np.float32)
        out[b] = acc.T
    return out


# revision 35
# speedup vs baseline: 1.5003x; 1.0136x over previous
"""Self-contained Trainium2 Bass kernel for multi-head causal attention with RoPE.

Problem: B=2, S=2048, D=2048, H=16 heads (HD=128), fp32 reference:
    q = rope(x @ wq.T), k = rope(x @ wk.T), v = x @ wv.T
    out = softmax(q k^T / sqrt(HD) + causal_mask) @ v @ wo.T

Sharding (8 cores): core c = (b, g) with b = c // 4 (batch), g = c % 4
(head-group of 4 heads).  Each core computes its head-group's attention for
its batch and a partial output projection (columns 512g:512g+512 of the
attention output times the matching wo rows).  The host sums the 4 partial
[D, S] tensors per batch and transposes back to [S, D].

On-chip layout is "transposed": Q^T/K^T are kept as [head_dim, seq] so the
QK^T matmul needs no transposes, scores come out as scoresT[k, q], and
probsT feeds the PV matmul directly (lhsT = V[sk, e]).  RoPE's even/odd pair
mixing becomes a half-partition mix via a PE matmul against a signed
permutation J (head dims of wq/wk are permuted host-side, which cancels in
q.k).  Causal masking: strictly-above-diagonal 128x512 score tiles are
skipped, diagonal tiles are computed at partial width (only columns at or
right of the diagonal) and get a multiplicative binary bf16 mask applied
after exp.  Softmax denominators: masked exp tiles are summed elementwise on
DVE (bf16, in place); the 128-partition reduction runs in f32 on GpSimd
(partition_all_reduce from the attn ucode library), followed by a DVE
reciprocal and the normalization multiply.
"""

import math

import numpy as np
import ml_dtypes

import concourse.bass as bass
import concourse.bacc as bacc
import concourse.mybir as mybir
from concourse.tile import TileContext
from concourse.bass_utils import run_bass_kernel_spmd
from concourse import library_config
from concourse.bass_isa import ReduceOp
from contextlib import ExitStack

B, S, D, H = 2, 2048, 2048, 16
HD = 128          # head dim
HPG = 4           # heads per core (group)
EG = HPG * HD     # 512 head dims per core
NCORES = 8
NSTRIP = 4        # q strips per sequence
STRIP = S // NSTRIP   # 512
SKT = 128         # k tile (partition dim of scoresT)
NDT = D // 128    # 16 contraction tiles for projections
SCALE = 1.0 / math.sqrt(HD)

BF16 = mybir.dt.bfloat16
F32 = mybir.dt.float32

LAST_EXEC_NS = None
LAST_RESULTS = None


def _build_program():
    nc = bacc.Bacc("TRN2", target_bir_lowering=False, debug=False,
                   num_devices=NCORES)
    xT_d = nc.dram_tensor("xT", [D, S], BF16, kind="ExternalInput").ap()
    wqT_d = nc.dram_tensor("wqT", [D, EG], BF16, kind="ExternalInput").ap()
    wkT_d = nc.dram_tensor("wkT", [D, EG], BF16, kind="ExternalInput").ap()
    wvT_d = nc.dram_tensor("wvT", [D, EG], BF16, kind="ExternalInput").ap()
    woT_d = nc.dram_tensor("woT", [EG, D], BF16, kind="ExternalInput").ap()
    cs_d = nc.dram_tensor("cs", [HD, S], BF16, kind="ExternalInput").ap()
    sn_d = nc.dram_tensor("sn", [HD, S], BF16, kind="ExternalInput").ap()
    mk_d = nc.dram_tensor("mk", [SKT, 4, STRIP], BF16, kind="ExternalInput").ap()
    jt_d = nc.dram_tensor("jt", [HD, HD], BF16, kind="ExternalInput").ap()
    outT_d = nc.dram_tensor("outT", [D, S], BF16, kind="ExternalOutput").ap()

    EXP = mybir.ActivationFunctionType.Exp

    with TileContext(nc) as tc, ExitStack() as ctx:
        nc.gpsimd.load_library(library_config.attn)

        wpool = ctx.enter_context(tc.tile_pool(name="wpool", bufs=1))
        kv = ctx.enter_context(tc.tile_pool(name="kv", bufs=1))
        xs = ctx.enter_context(tc.tile_pool(name="xs", bufs=2))
        qs = ctx.enter_context(tc.tile_pool(name="qs", bufs=2))
        rp = ctx.enter_context(tc.tile_pool(name="rp", bufs=3))
        ep = ctx.enter_context(tc.tile_pool(name="ep", bufs=5))
        ot = ctx.enter_context(tc.tile_pool(name="ot", bufs=2))
        po = ctx.enter_context(tc.tile_pool(name="po", bufs=5))
        nrm = ctx.enter_context(tc.tile_pool(name="nrm", bufs=2))
        rcbp = ctx.enter_context(tc.tile_pool(name="rcbp", bufs=2))
        scps = ctx.enter_context(tc.tile_pool(name="scps", bufs=2, space="PSUM"))
        prps = ctx.enter_context(tc.tile_pool(name="prps", bufs=2, space="PSUM"))
        pvps = ctx.enter_context(tc.tile_pool(name="pvps", bufs=2, space="PSUM"))
        wops = ctx.enter_context(tc.tile_pool(name="wops", bufs=2, space="PSUM"))

        # persistent SBUF tensors
        wq_sb = wpool.tile([128, NDT, EG], BF16)
        wk_sb = wpool.tile([128, NDT, EG], BF16)
        wv_sb = wpool.tile([128, NDT, EG], BF16)
        wo_sb = wpool.tile([128, HPG, D], BF16)
        cs_sb = wpool.tile([128, S], BF16)
        sn_sb = wpool.tile([128, S], BF16)
        mk_sb = wpool.tile([128, 4, STRIP], BF16)
        jt_sb = wpool.tile([HD, HD], BF16)
        KT_sb = kv.tile([128, HPG, S], BF16)       # [e, h, sk] rope'd K^T
        V_sb = kv.tile([128, S // 128, EG], BF16)  # [sk, sk_tile, e]

        xt0 = xs.tile([128, NDT, STRIP], BF16, tag="xt")
        x0r = xT_d[:, 0:STRIP].rearrange("(t p) s -> p t s", p=128)

        # --- DMA issue order tuned for the critical path: the first q
        # projection needs wq chunk 0 + the first x d-tiles.
        wqr = wqT_d.rearrange("(t p) e -> p t e", p=128)
        wkr = wkT_d.rearrange("(t p) e -> p t e", p=128)
        wvr = wvT_d.rearrange("(t p) e -> p t e", p=128)
        nc.sync.dma_start(out=wq_sb[:, 0:2, :], in_=wqr[:, 0:2, :])
        nc.sync.dma_start(out=xt0[:, 0:2, :], in_=x0r[:, 0:2, :])
        nc.sync.dma_start(out=wq_sb[:, 2:4, :], in_=wqr[:, 2:4, :])
        nc.sync.dma_start(out=xt0[:, 2:4, :], in_=x0r[:, 2:4, :])
        for c0 in range(4, NDT, 4):
            nc.sync.dma_start(out=wq_sb[:, c0:c0 + 4, :], in_=wqr[:, c0:c0 + 4, :])
            nc.sync.dma_start(out=xt0[:, c0:c0 + 4, :], in_=x0r[:, c0:c0 + 4, :])
        for c0 in range(0, NDT, 4):
            nc.sync.dma_start(out=wk_sb[:, c0:c0 + 4, :], in_=wkr[:, c0:c0 + 4, :])
        nc.sync.dma_start(out=cs_sb, in_=cs_d)
        nc.sync.dma_start(out=sn_sb, in_=sn_d)
        nc.sync.dma_start(out=jt_sb, in_=jt_d)
        xt1 = xs.tile([128, NDT, STRIP], BF16, tag="xt")
        nc.sync.dma_start(
            out=xt1,
            in_=xT_d[:, STRIP:2 * STRIP].rearrange("(t p) s -> p t s", p=128))
        nc.sync.dma_start(out=mk_sb, in_=mk_d)
        for c0 in range(0, NDT, 4):
            nc.sync.dma_start(out=wv_sb[:, c0:c0 + 4, :], in_=wvr[:, c0:c0 + 4, :])
        nc.sync.dma_start(out=wo_sb, in_=woT_d.rearrange("(t p) n -> p t n", p=128))

        for j in range(NSTRIP):
            s0 = j * STRIP
            if j == 0:
                xt = xt0
            elif j == 1:
                xt = xt1
            else:
                xt = xs.tile([128, NDT, STRIP], BF16, tag="xt")
                nc.sync.dma_start(
                    out=xt,
                    in_=xT_d[:, s0:s0 + STRIP].rearrange("(t p) s -> p t s", p=128))
            qt = qs.tile([128, HPG, STRIP], BF16, tag="qt")

            # --- projections + RoPE for this strip ---
            if j == 0:
                # strip 0: chunk-outer over the 4-dt DMA chunks so the first
                # matmuls start as soon as (wq chunk 0, x chunk 0) land; the
                # 4 concurrent head accumulators borrow the idle sc psum bufs
                qp0 = prps.tile([128, STRIP], F32, tag="mm")
                qp1 = prps.tile([128, STRIP], F32, tag="mm")
                qp2 = scps.tile([128, STRIP], F32, tag="sc")
                qp3 = scps.tile([128, STRIP], F32, tag="sc")
                qps0 = [qp0, qp1, qp2, qp3]
                for c0, cw in ((0, 2), (2, 2), (4, 4), (8, 4), (12, 4)):
                    for h in range(HPG):
                        e0 = h * HD
                        for dt in range(c0, c0 + cw):
                            nc.tensor.matmul(qps0[h],
                                             lhsT=wq_sb[:, dt, e0:e0 + HD],
                                             rhs=xt[:, dt, :],
                                             start=(dt == 0),
                                             stop=(dt == NDT - 1))
            for h in range(HPG):
                e0 = h * HD
                if j == 0:
                    q_ps = qps0[h]
                else:
                    q_ps = prps.tile([128, STRIP], F32, tag="mm")
                    for dt in range(NDT):
                        nc.tensor.matmul(q_ps, lhsT=wq_sb[:, dt, e0:e0 + HD],
                                         rhs=xt[:, dt, :],
                                         start=(dt == 0), stop=(dt == NDT - 1))
                q_sb = rp.tile([128, STRIP], BF16, tag="qsb")
                nc.scalar.copy(q_sb, q_ps)
                if j == 0:
                    jq_ps = pvps.tile([128, STRIP], F32, tag="pv")
                else:
                    jq_ps = prps.tile([128, STRIP], F32, tag="mm")
                nc.tensor.matmul(jq_ps, lhsT=jt_sb, rhs=q_sb,
                                 start=True, stop=True)
                t1 = rp.tile([128, STRIP], BF16, tag="ra")
                nc.vector.tensor_mul(t1, q_sb, cs_sb[:, s0:s0 + STRIP])
                t2 = rp.tile([128, STRIP], BF16, tag="rb")
                nc.vector.tensor_mul(t2, jq_ps, sn_sb[:, s0:s0 + STRIP])
                nc.vector.tensor_add(qt[:, h, :], t1, t2)

                k_ps = prps.tile([128, STRIP], F32, tag="mm")
                for dt in range(NDT):
                    nc.tensor.matmul(k_ps, lhsT=wk_sb[:, dt, e0:e0 + HD],
                                     rhs=xt[:, dt, :],
                                     start=(dt == 0), stop=(dt == NDT - 1))
                k_sb = rp.tile([128, STRIP], BF16, tag="qsb")
                nc.scalar.copy(k_sb, k_ps)
                jk_ps = prps.tile([128, STRIP], F32, tag="mm")
                nc.tensor.matmul(jk_ps, lhsT=jt_sb, rhs=k_sb,
                                 start=True, stop=True)
                t3 = rp.tile([128, STRIP], BF16, tag="ra")
                nc.vector.tensor_mul(t3, k_sb, cs_sb[:, s0:s0 + STRIP])
                t4 = rp.tile([128, STRIP], BF16, tag="rb")
                nc.vector.tensor_mul(t4, jk_ps, sn_sb[:, s0:s0 + STRIP])
                nc.vector.tensor_add(KT_sb[:, h, s0:s0 + STRIP], t3, t4)

            for st in range(4):
                v_ps = prps.tile([128, EG], F32, tag="mm")
                for dt in range(NDT):
                    nc.tensor.matmul(v_ps, lhsT=xt[:, dt, st * 128:(st + 1) * 128],
                                     rhs=wv_sb[:, dt, :],
                                     start=(dt == 0), stop=(dt == NDT - 1))
                nc.scalar.copy(V_sb[:, j * 4 + st, :], v_ps)

            # --- attention for this strip ---
            otile = ot.tile([128, HPG, STRIP], BF16, tag="ot")
            nsk = 4 * j + 4
            for h in range(HPG):
                e0 = h * HD
                pv_ps = pvps.tile([128, STRIP], F32, tag="pv")
                acc = None   # bf16 running elementwise sum of masked exps;
                #              the 128-partition reduction happens in f32 on
                #              Pool, so bf16 chain rounding averages out
                for skt in range(nsk):
                    d = skt - 4 * j   # >= 0 on the diagonal block
                    w = STRIP - 128 * d if d > 0 else STRIP
                    sc_ps = scps.tile([128, STRIP], F32, tag="sc")
                    nc.tensor.matmul(sc_ps[:, 0:w],
                                     lhsT=KT_sb[:, h, skt * 128:(skt + 1) * 128],
                                     rhs=qt[:, h, STRIP - w:STRIP],
                                     start=True, stop=True)
                    ex = ep.tile([128, STRIP], BF16, tag="ex")
                    nc.scalar.activation(ex[:, 0:w], sc_ps[:, 0:w], EXP,
                                         scale=SCALE)
                    if d >= 0:
                        exm = ep.tile([128, STRIP], BF16, tag="exm")
                        nc.vector.tensor_mul(exm[:, 0:w], ex[:, 0:w],
                                             mk_sb[:, d, STRIP - w:STRIP])
                    else:
                        exm = ex
                    nc.tensor.matmul(pv_ps[:, STRIP - w:STRIP],
                                     lhsT=V_sb[:, skt, e0:e0 + HD],
                                     rhs=exm[:, 0:w], start=(skt == 0),
                                     stop=(skt == nsk - 1))
                    if skt == 0:
                        acc = exm      # full width; later adds are in-place
                    else:
                        nc.vector.tensor_add(acc[:, STRIP - w:STRIP],
                                             acc[:, STRIP - w:STRIP],
                                             exm[:, 0:w])
                den = nrm.tile([128, STRIP], F32, tag="den")
                nc.gpsimd.partition_all_reduce(den, acc, 128, ReduceOp.add)
                rcb = rcbp.tile([128, STRIP], F32, tag="rcb")
                nc.vector.reciprocal(rcb, den)
                nc.vector.tensor_mul(otile[:, h, :], pv_ps, rcb)

            # --- partial output projection for this strip ---
            for nt in range(NDT):
                if j == NSTRIP - 1 and nt % 4 == 3:
                    pr = scps.tile([128, STRIP], F32, tag="sc")
                elif j == NSTRIP - 1 and nt % 4 == 1:
                    pr = prps.tile([128, STRIP], F32, tag="mm")
                else:
                    pr = wops.tile([128, STRIP], F32, tag="pr")
                for h in range(HPG):
                    nc.tensor.matmul(pr, lhsT=wo_sb[:, h, nt * 128:(nt + 1) * 128],
                                     rhs=otile[:, h, :],
                                     start=(h == 0), stop=(h == HPG - 1))
                pr_sb = po.tile([128, STRIP], BF16, tag="po")
                if j == NSTRIP - 1 and nt % 2 == 1:
                    nc.scalar.copy(pr_sb, pr)
                else:
                    nc.vector.tensor_copy(pr_sb, pr)
                nc.sync.dma_start(
                    out=outT_d[nt * 128:(nt + 1) * 128, s0:s0 + STRIP], in_=pr_sb)

    return nc


_PERM = np.concatenate([np.arange(0, HD, 2), np.arange(1, HD, 2)])


def _host_prep(x, wq, wk, wv, wo, freqs_cos, freqs_sin, mask):
    bf16 = ml_dtypes.bfloat16
    x = np.asarray(x, np.float32)
    wq = np.asarray(wq, np.float32)
    wk = np.asarray(wk, np.float32)
    wv = np.asarray(wv, np.float32)
    wo = np.asarray(wo, np.float32)
    cos = np.asarray(freqs_cos, np.float32)   # [S, HD/2]
    sin = np.asarray(freqs_sin, np.float32)
    mask = np.asarray(mask, np.float32)

    cosH = cos.T                               # [64, S]
    sinH = sin.T
    cs = np.vstack([cosH, cosH]).astype(bf16)  # [128, S]
    sn = np.vstack([sinH, sinH]).astype(bf16)

    # multiplicative binary causal mask for the 4 diagonal-tile flavours:
    # mk[k, d, q] = 1 where allowed, 0 where masked
    mk = np.empty((SKT, 4, STRIP), np.float32)
    for d_ in range(4):
        sub = mask[0:STRIP, d_ * SKT:(d_ + 1) * SKT]   # [q, k]
        mk[:, d_, :] = np.where(np.isfinite(sub), 1.0, 0.0).T
    mk = mk.astype(bf16)

    perm_g = np.concatenate([h * HD + _PERM for h in range(HPG)])

    # lhsT of the rope pair-mix matmul: (J q) rows 0:64 = -q[64:128],
    # rows 64:128 = +q[0:64]; jt = J.T
    jt = np.zeros((HD, HD), np.float32)
    jt[np.arange(64), np.arange(64) + 64] = 1.0
    jt[np.arange(64) + 64, np.arange(64)] = -1.0
    jt = jt.astype(bf16)

    in_maps = []
    for c in range(NCORES):
        b, g = c // HPG, c % HPG
        rows = slice(g * EG, (g + 1) * EG)
        wq_g = wq[rows][perm_g]                # [EG, D], head dims permuted
        wk_g = wk[rows][perm_g]
        wv_g = wv[rows]
        in_maps.append({
            "xT": np.ascontiguousarray(x[b].T).astype(bf16),
            "wqT": np.ascontiguousarray(wq_g.T).astype(bf16),
            "wkT": np.ascontiguousarray(wk_g.T).astype(bf16),
            "wvT": np.ascontiguousarray(wv_g.T).astype(bf16),
            "woT": np.ascontiguousarray(wo[:, rows].T).astype(bf16),
            "cs": cs, "sn": sn, "mk": mk, "jt": jt,
        })
    return in_maps


def kernel(x, wq, wk, wv, wo, freqs_cos, freqs_sin, mask, start_pos):
    global LAST_EXEC_NS, LAST_RESULTS
    in_maps = _host_prep(x, wq, wk, wv, wo, freqs_cos, freqs_sin, mask)
    nc = _build_program()
    nc.finalize()
    res = run_bass_kernel_spmd(nc, in_maps, core_ids=list(range(NCORES)),
                               trace=False)
    LAST_EXEC_NS = res.exec_time_ns
    LAST_RESULTS = res
    out = np.empty((B, S, D), np.float32)
    for b in range(B):
        acc = np.zeros((D, S), np.float32)
        for g in range(HPG):
            acc += res.results[b * HPG + g]["outT"].astype(np.float32)
        out[b] = acc.T
    return out


# revision 38
# speedup vs baseline: 1.5018x; 1.0010x over previous
"""Self-contained Trainium2 Bass kernel for multi-head causal attention with RoPE.

Problem: B=2, S=2048, D=2048, H=16 heads (HD=128), fp32 reference:
    q = rope(x @ wq.T), k = rope(x @ wk.T), v = x @ wv.T
    out = softmax(q k^T / sqrt(HD) + causal_mask) @ v @ wo.T

Sharding (8 cores): core c = (b, g) with b = c // 4 (batch), g = c % 4
(head-group of 4 heads).  Each core computes its head-group's attention for
its batch and a partial output projection (columns 512g:512g+512 of the
attention output times the matching wo rows).  The host sums the 4 partial
[D, S] tensors per batch and transposes back to [S, D].

On-chip layout is "transposed": Q^T/K^T are kept as [head_dim, seq] so the
QK^T matmul needs no transposes, scores come out as scoresT[k, q], and
probsT feeds the PV matmul directly (lhsT = V[sk, e]).  RoPE's even/odd pair
mixing becomes a half-partition mix via a PE matmul against a signed
permutation J (head dims of wq/wk are permuted host-side, which cancels in
q.k).  Causal masking: strictly-above-diagonal 128x512 score tiles are
skipped, diagonal tiles are computed at partial width (only columns at or
right of the diagonal) and get a multiplicative binary bf16 mask applied
after exp.  Softmax denominators: masked exp tiles are summed elementwise on
DVE (bf16, in place); the 128-partition reduction runs in f32 on GpSimd
(partition_all_reduce from the attn ucode library), followed by a DVE
reciprocal and the normalization multiply.
"""

import math

import numpy as np
import ml_dtypes

import concourse.bass as bass
import concourse.bacc as bacc
import concourse.mybir as mybir
from concourse.tile import TileContext
from concourse.bass_utils import run_bass_kernel_spmd
from concourse import library_config
from concourse.bass_isa import ReduceOp
from contextlib import ExitStack

B, S, D, H = 2, 2048, 2048, 16
HD = 128          # head dim
HPG = 4           # heads per core (group)
EG = HPG * HD     # 512 head dims per core
NCORES = 8
NSTRIP = 4        # q strips per sequence
STRIP = S // NSTRIP   # 512
SKT = 128         # k tile (partition dim of scoresT)
NDT = D // 128    # 16 contraction tiles for projections
SCALE = 1.0 / math.sqrt(HD)

BF16 = mybir.dt.bfloat16
F32 = mybir.dt.float32

LAST_EXEC_NS = None
LAST_RESULTS = None


def _build_program():
    nc = bacc.Bacc("TRN2", target_bir_lowering=False, debug=False,
                   num_devices=NCORES)
    xT_d = nc.dram_tensor("xT", [D, S], BF16, kind="ExternalInput").ap()
    wqT_d = nc.dram_tensor("wqT", [D, EG], BF16, kind="ExternalInput").ap()
    wkT_d = nc.dram_tensor("wkT", [D, EG], BF16, kind="ExternalInput").ap()
    wvT_d = nc.dram_tensor("wvT", [D, EG], BF16, kind="ExternalInput").ap()
    woT_d = nc.dram_tensor("woT", [EG, D], BF16, kind="ExternalInput").ap()
    cs_d = nc.dram_tensor("cs", [HD, S], BF16, kind="ExternalInput").ap()
    sn_d = nc.dram_tensor("sn", [HD, S], BF16, kind="ExternalInput").ap()
    mk_d = nc.dram_tensor("mk", [SKT, 4, STRIP], BF16, kind="ExternalInput").ap()
    jt_d = nc.dram_tensor("jt", [HD, HD], BF16, kind="ExternalInput").ap()
    outT_d = nc.dram_tensor("outT", [D, S], BF16, kind="ExternalOutput").ap()

    EXP = mybir.ActivationFunctionType.Exp

    with TileContext(nc) as tc, ExitStack() as ctx:
        nc.gpsimd.load_library(library_config.attn)

        wpool = ctx.enter_context(tc.tile_pool(name="wpool", bufs=1))
        kv = ctx.enter_context(tc.tile_pool(name="kv", bufs=1))
        xs = ctx.enter_context(tc.tile_pool(name="xs", bufs=2))
        qs = ctx.enter_context(tc.tile_pool(name="qs", bufs=2))
        rp = ctx.enter_context(tc.tile_pool(name="rp", bufs=3))
        ep = ctx.enter_context(tc.tile_pool(name="ep", bufs=5))
        ot = ctx.enter_context(tc.tile_pool(name="ot", bufs=2))
        po = ctx.enter_context(tc.tile_pool(name="po", bufs=5))
        nrm = ctx.enter_context(tc.tile_pool(name="nrm", bufs=2))
        rcbp = ctx.enter_context(tc.tile_pool(name="rcbp", bufs=2))
        scps = ctx.enter_context(tc.tile_pool(name="scps", bufs=2, space="PSUM"))
        prps = ctx.enter_context(tc.tile_pool(name="prps", bufs=2, space="PSUM"))
        pvps = ctx.enter_context(tc.tile_pool(name="pvps", bufs=2, space="PSUM"))
        wops = ctx.enter_context(tc.tile_pool(name="wops", bufs=2, space="PSUM"))

        # persistent SBUF tensors
        wq_sb = wpool.tile([128, NDT, EG], BF16)
        wk_sb = wpool.tile([128, NDT, EG], BF16)
        wv_sb = wpool.tile([128, NDT, EG], BF16)
        wo_sb = wpool.tile([128, HPG, D], BF16)
        cs_sb = wpool.tile([128, S], BF16)
        sn_sb = wpool.tile([128, S], BF16)
        mk_sb = wpool.tile([128, 4, STRIP], BF16)
        jt_sb = wpool.tile([HD, HD], BF16)
        KT_sb = kv.tile([128, HPG, S], BF16)       # [e, h, sk] rope'd K^T
        V_sb = kv.tile([128, S // 128, EG], BF16)  # [sk, sk_tile, e]

        xt0 = xs.tile([128, NDT, STRIP], BF16, tag="xt")
        x0r = xT_d[:, 0:STRIP].rearrange("(t p) s -> p t s", p=128)

        # --- DMA issue order tuned for the critical path: the first q
        # projection needs wq chunk 0 + the first x d-tiles.
        wqr = wqT_d.rearrange("(t p) e -> p t e", p=128)
        wkr = wkT_d.rearrange("(t p) e -> p t e", p=128)
        wvr = wvT_d.rearrange("(t p) e -> p t e", p=128)
        nc.sync.dma_start(out=wq_sb[:, 0:2, :], in_=wqr[:, 0:2, :])
        nc.sync.dma_start(out=xt0[:, 0:2, :], in_=x0r[:, 0:2, :])
        nc.sync.dma_start(out=wq_sb[:, 2:4, :], in_=wqr[:, 2:4, :])
        nc.sync.dma_start(out=xt0[:, 2:4, :], in_=x0r[:, 2:4, :])
        for c0 in range(4, NDT, 4):
            nc.sync.dma_start(out=wq_sb[:, c0:c0 + 4, :], in_=wqr[:, c0:c0 + 4, :])
            nc.sync.dma_start(out=xt0[:, c0:c0 + 4, :], in_=x0r[:, c0:c0 + 4, :])
        for c0 in range(0, NDT, 4):
            nc.sync.dma_start(out=wk_sb[:, c0:c0 + 4, :], in_=wkr[:, c0:c0 + 4, :])
        nc.sync.dma_start(out=cs_sb, in_=cs_d)
        nc.sync.dma_start(out=sn_sb, in_=sn_d)
        nc.sync.dma_start(out=jt_sb, in_=jt_d)
        xt1 = xs.tile([128, NDT, STRIP], BF16, tag="xt")
        nc.sync.dma_start(
            out=xt1,
            in_=xT_d[:, STRIP:2 * STRIP].rearrange("(t p) s -> p t s", p=128))
        nc.sync.dma_start(out=mk_sb, in_=mk_d)
        for c0 in range(0, NDT, 4):
            nc.sync.dma_start(out=wv_sb[:, c0:c0 + 4, :], in_=wvr[:, c0:c0 + 4, :])
        nc.sync.dma_start(out=wo_sb, in_=woT_d.rearrange("(t p) n -> p t n", p=128))

        for j in range(NSTRIP):
            s0 = j * STRIP
            if j == 0:
                xt = xt0
            elif j == 1:
                xt = xt1
            else:
                xt = xs.tile([128, NDT, STRIP], BF16, tag="xt")
                nc.sync.dma_start(
                    out=xt,
                    in_=xT_d[:, s0:s0 + STRIP].rearrange("(t p) s -> p t s", p=128))
            qt = qs.tile([128, HPG, STRIP], BF16, tag="qt")

            # --- projections + RoPE for this strip ---
            if j == 0:
                # strip 0: chunk-outer over the 4-dt DMA chunks so the first
                # matmuls start as soon as (wq chunk 0, x chunk 0) land; the
                # 4 concurrent head accumulators borrow the idle sc psum bufs
                qp0 = prps.tile([128, STRIP], F32, tag="mm")
                qp1 = prps.tile([128, STRIP], F32, tag="mm")
                qp2 = scps.tile([128, STRIP], F32, tag="sc")
                qp3 = scps.tile([128, STRIP], F32, tag="sc")
                qps0 = [qp0, qp1, qp2, qp3]
                for c0, cw in ((0, 2), (2, 2), (4, 4), (8, 4), (12, 4)):
                    for h in range(HPG):
                        e0 = h * HD
                        for dt in range(c0, c0 + cw):
                            nc.tensor.matmul(qps0[h],
                                             lhsT=wq_sb[:, dt, e0:e0 + HD],
                                             rhs=xt[:, dt, :],
                                             start=(dt == 0),
                                             stop=(dt == NDT - 1))
            for h in range(HPG):
                e0 = h * HD
                if j == 0:
                    q_ps = qps0[h]
                else:
                    q_ps = prps.tile([128, STRIP], F32, tag="mm")
                    for dt in range(NDT):
                        nc.tensor.matmul(q_ps, lhsT=wq_sb[:, dt, e0:e0 + HD],
                                         rhs=xt[:, dt, :],
                                         start=(dt == 0), stop=(dt == NDT - 1))
                q_sb = rp.tile([128, STRIP], BF16, tag="qsb")
                nc.scalar.copy(q_sb, q_ps)
                if j == 0:
                    jq_ps = pvps.tile([128, STRIP], F32, tag="pv")
                else:
                    jq_ps = prps.tile([128, STRIP], F32, tag="mm")
                nc.tensor.matmul(jq_ps, lhsT=jt_sb, rhs=q_sb,
                                 start=True, stop=True)
                jq_sb = rp.tile([128, STRIP], BF16, tag="jsb")
                nc.scalar.copy(jq_sb, jq_ps)
                t1 = rp.tile([128, STRIP], BF16, tag="ra")
                nc.vector.tensor_mul(t1, q_sb, cs_sb[:, s0:s0 + STRIP])
                t2 = rp.tile([128, STRIP], BF16, tag="rb")
                nc.vector.tensor_mul(t2, jq_sb, sn_sb[:, s0:s0 + STRIP])
                nc.vector.tensor_add(qt[:, h, :], t1, t2)

                k_ps = prps.tile([128, STRIP], F32, tag="mm")
                for dt in range(NDT):
                    nc.tensor.matmul(k_ps, lhsT=wk_sb[:, dt, e0:e0 + HD],
                                     rhs=xt[:, dt, :],
                                     start=(dt == 0), stop=(dt == NDT - 1))
                k_sb = rp.tile([128, STRIP], BF16, tag="qsb")
                nc.scalar.copy(k_sb, k_ps)
                jk_ps = prps.tile([128, STRIP], F32, tag="mm")
                nc.tensor.matmul(jk_ps, lhsT=jt_sb, rhs=k_sb,
                                 start=True, stop=True)
                jk_sb = rp.tile([128, STRIP], BF16, tag="jsb")
                nc.scalar.copy(jk_sb, jk_ps)
                t3 = rp.tile([128, STRIP], BF16, tag="ra")
                nc.vector.tensor_mul(t3, k_sb, cs_sb[:, s0:s0 + STRIP])
                t4 = rp.tile([128, STRIP], BF16, tag="rb")
                nc.vector.tensor_mul(t4, jk_sb, sn_sb[:, s0:s0 + STRIP])
                nc.vector.tensor_add(KT_sb[:, h, s0:s0 + STRIP], t3, t4)

            for st in range(4):
                v_ps = prps.tile([128, EG], F32, tag="mm")
                for dt in range(NDT):
                    nc.tensor.matmul(v_ps, lhsT=xt[:, dt, st * 128:(st + 1) * 128],
                                     rhs=wv_sb[:, dt, :],
                                     start=(dt == 0), stop=(dt == NDT - 1))
                nc.scalar.copy(V_sb[:, j * 4 + st, :], v_ps)

            # --- attention for this strip ---
            otile = ot.tile([128, HPG, STRIP], BF16, tag="ot")
            nsk = 4 * j + 4
            for h in range(HPG):
                e0 = h * HD
                pv_ps = pvps.tile([128, STRIP], F32, tag="pv")
                acc = None   # bf16 running elementwise sum of masked exps;
                #              the 128-partition reduction happens in f32 on
                #              Pool, so bf16 chain rounding averages out
                for skt in range(nsk):
                    d = skt - 4 * j   # >= 0 on the diagonal block
                    w = STRIP - 128 * d if d > 0 else STRIP
                    sc_ps = scps.tile([128, STRIP], F32, tag="sc")
                    nc.tensor.matmul(sc_ps[:, 0:w],
                                     lhsT=KT_sb[:, h, skt * 128:(skt + 1) * 128],
                                     rhs=qt[:, h, STRIP - w:STRIP],
                                     start=True, stop=True)
                    ex = ep.tile([128, STRIP], BF16, tag="ex")
                    nc.scalar.activation(ex[:, 0:w], sc_ps[:, 0:w], EXP,
                                         scale=SCALE)
                    if d >= 0:
                        exm = ep.tile([128, STRIP], BF16, tag="exm")
                        nc.vector.tensor_mul(exm[:, 0:w], ex[:, 0:w],
                                             mk_sb[:, d, STRIP - w:STRIP])
                    else:
                        exm = ex
                    nc.tensor.matmul(pv_ps[:, STRIP - w:STRIP],
                                     lhsT=V_sb[:, skt, e0:e0 + HD],
                                     rhs=exm[:, 0:w], start=(skt == 0),
                                     stop=(skt == nsk - 1))
                    if skt == 0:
                        acc = exm      # full width; later adds are in-place
                    else:
                        nc.vector.tensor_add(acc[:, STRIP - w:STRIP],
                                             acc[:, STRIP - w:STRIP],
                                             exm[:, 0:w])
                den = nrm.tile([128, STRIP], F32, tag="den")
                nc.gpsimd.partition_all_reduce(den, acc, 128, ReduceOp.add)
                rcb = rcbp.tile([128, STRIP], F32, tag="rcb")
                nc.vector.reciprocal(rcb, den)
                nc.vector.tensor_mul(otile[:, h, :], pv_ps, rcb)

            # --- partial output projection for this strip ---
            for nt in range(NDT):
                if j == NSTRIP - 1 and nt % 4 == 3:
                    pr = scps.tile([128, STRIP], F32, tag="sc")
                elif j == NSTRIP - 1 and nt % 4 == 1:
                    pr = prps.tile([128, STRIP], F32, tag="mm")
                else:
                    pr = wops.tile([128, STRIP], F32, tag="pr")
                for h in range(HPG):
                    nc.tensor.matmul(pr, lhsT=wo_sb[:, h, nt * 128:(nt + 1) * 128],
                                     rhs=otile[:, h, :],
                                     start=(h == 0), stop=(h == HPG - 1))
                pr_sb = po.tile([128, STRIP], BF16, tag="po")
                if j == NSTRIP - 1 and nt % 2 == 1:
                    nc.scalar.copy(pr_sb, pr)
                else:
                    nc.vector.tensor_copy(pr_sb, pr)
                nc.sync.dma_start(
                    out=outT_d[nt * 128:(nt + 1) * 128, s0:s0 + STRIP], in_=pr_sb)

    return nc


_PERM = np.concatenate([np.arange(0, HD, 2), np.arange(1, HD, 2)])


def _host_prep(x, wq, wk, wv, wo, freqs_cos, freqs_sin, mask):
    bf16 = ml_dtypes.bfloat16
    x = np.asarray(x, np.float32)
    wq = np.asarray(wq, np.float32)
    wk = np.asarray(wk, np.float32)
    wv = np.asarray(wv, np.float32)
    wo = np.asarray(wo, np.float32)
    cos = np.asarray(freqs_cos, np.float32)   # [S, HD/2]
    sin = np.asarray(freqs_sin, np.float32)
    mask = np.asarray(mask, np.float32)

    cosH = cos.T                               # [64, S]
    sinH = sin.T
    cs = np.vstack([cosH, cosH]).astype(bf16)  # [128, S]
    sn = np.vstack([sinH, sinH]).astype(bf16)

    # multiplicative binary causal mask for the 4 diagonal-tile flavours:
    # mk[k, d, q] = 1 where allowed, 0 where masked
    mk = np.empty((SKT, 4, STRIP), np.float32)
    for d_ in range(4):
        sub = mask[0:STRIP, d_ * SKT:(d_ + 1) * SKT]   # [q, k]
        mk[:, d_, :] = np.where(np.isfinite(sub), 1.0, 0.0).T
    mk = mk.astype(bf16)

    perm_g = np.concatenate([h * HD + _PERM for h in range(HPG)])

    # lhsT of the rope pair-mix matmul: (J q) rows 0:64 = -q[64:128],
    # rows 64:128 = +q[0:64]; jt = J.T
    jt = np.zeros((HD, HD), np.float32)
    jt[np.arange(64), np.arange(64) + 64] = 1.0
    jt[np.arange(64) + 64, np.arange(64)] = -1.0
    jt = jt.astype(bf16)

    in_maps = []
    for c in range(NCORES):
        b, g = c // HPG, c % HPG
        rows = slice(g * EG, (g + 1) * EG)
        wq_g = wq[rows][perm_g]                # [EG, D], head dims permuted
        wk_g = wk[rows][perm_g]
        wv_g = wv[rows]
        in_maps.append({
            "xT": np.ascontiguousarray(x[b].T).astype(bf16),
            "wqT": np.ascontiguousarray(wq_g.T).astype(bf16),
            "wkT": np.ascontiguousarray(wk_g.T).astype(bf16),
            "wvT": np.ascontiguousarray(wv_g.T).astype(bf16),
            "woT": np.ascontiguousarray(wo[:, rows].T).astype(bf16),
            "cs": cs, "sn": sn, "mk": mk, "jt": jt,
        })
    return in_maps


def kernel(x, wq, wk, wv, wo, freqs_cos, freqs_sin, mask, start_pos):
    global LAST_EXEC_NS, LAST_RESULTS
    in_maps = _host_prep(x, wq, wk, wv, wo, freqs_cos, freqs_sin, mask)
    nc = _build_program()
    nc.finalize()
    res = run_bass_kernel_spmd(nc, in_maps, core_ids=list(range(NCORES)),
                               trace=False)
    LAST_EXEC_NS = res.exec_time_ns
    LAST_RESULTS = res
    out = np.empty((B, S, D), np.float32)
    for b in range(B):
        acc = np.zeros((D, S), np.float32)
        for g in range(HPG):
            acc += res.results[b * HPG + g]["outT"].astype(np.float32)
        out[b] = acc.T
    return out


# revision 47
# speedup vs baseline: 1.5133x; 1.0077x over previous
"""Self-contained Trainium2 Bass kernel for multi-head causal attention with RoPE.

Problem: B=2, S=2048, D=2048, H=16 heads (HD=128), fp32 reference:
    q = rope(x @ wq.T), k = rope(x @ wk.T), v = x @ wv.T
    out = softmax(q k^T / sqrt(HD) + causal_mask) @ v @ wo.T

Sharding (8 cores): core c = (b, g) with b = c // 4 (batch), g = c % 4
(head-group of 4 heads).  Each core computes its head-group's attention for
its batch and a partial output projection (columns 512g:512g+512 of the
attention output times the matching wo rows).  The host sums the 4 partial
[D, S] tensors per batch and transposes back to [S, D].

On-chip layout is "transposed": Q^T/K^T are kept as [head_dim, seq] so the
QK^T matmul needs no transposes, scores come out as scoresT[k, q], and
probsT feeds the PV matmul directly (lhsT = V[sk, e]).  RoPE's even/odd pair
mixing becomes a half-partition mix via a PE matmul against a signed
permutation J (head dims of wq/wk are permuted host-side, which cancels in
q.k).  Causal masking: strictly-above-diagonal 128x512 score tiles are
skipped, diagonal tiles are computed at partial width (only columns at or
right of the diagonal) and get a multiplicative binary bf16 mask applied
after exp.  Softmax denominators: masked exp tiles are summed elementwise on
DVE (bf16, in place); the 128-partition reduction runs in f32 on GpSimd
(partition_all_reduce from the attn ucode library), followed by a DVE
reciprocal and the normalization multiply.
"""

import math

import numpy as np
import ml_dtypes

import concourse.bass as bass
import concourse.bacc as bacc
import concourse.mybir as mybir
from concourse.tile import TileContext
from concourse.bass_utils import run_bass_kernel_spmd
from concourse import library_config
from concourse.bass_isa import ReduceOp
from contextlib import ExitStack

B, S, D, H = 2, 2048, 2048, 16
HD = 128          # head dim
HPG = 4           # heads per core (group)
EG = HPG * HD     # 512 head dims per core
NCORES = 8
NSTRIP = 4        # q strips per sequence
STRIP = S // NSTRIP   # 512
SKT = 128         # k tile (partition dim of scoresT)
NDT = D // 128    # 16 contraction tiles for projections
SCALE = 1.0 / math.sqrt(HD)

BF16 = mybir.dt.bfloat16
F32 = mybir.dt.float32

LAST_EXEC_NS = None
LAST_RESULTS = None


def _build_program():
    nc = bacc.Bacc("TRN2", target_bir_lowering=False, debug=False,
                   num_devices=NCORES)
    xT_d = nc.dram_tensor("xT", [D, S], BF16, kind="ExternalInput").ap()
    wqT_d = nc.dram_tensor("wqT", [D, EG], BF16, kind="ExternalInput").ap()
    wkT_d = nc.dram_tensor("wkT", [D, EG], BF16, kind="ExternalInput").ap()
    wvT_d = nc.dram_tensor("wvT", [D, EG], BF16, kind="ExternalInput").ap()
    woT_d = nc.dram_tensor("woT", [EG, D], BF16, kind="ExternalInput").ap()
    cs_d = nc.dram_tensor("cs", [HD, S], BF16, kind="ExternalInput").ap()
    sn_d = nc.dram_tensor("sn", [HD, S], BF16, kind="ExternalInput").ap()
    mk_d = nc.dram_tensor("mk", [SKT, 4, STRIP], BF16, kind="ExternalInput").ap()
    jt_d = nc.dram_tensor("jt", [HD, HD], BF16, kind="ExternalInput").ap()
    outT_d = nc.dram_tensor("outT", [D, S], BF16, kind="ExternalOutput").ap()

    EXP = mybir.ActivationFunctionType.Exp

    with TileContext(nc) as tc, ExitStack() as ctx:
        nc.gpsimd.load_library(library_config.attn)

        wpool = ctx.enter_context(tc.tile_pool(name="wpool", bufs=1))
        kv = ctx.enter_context(tc.tile_pool(name="kv", bufs=1))
        xs = ctx.enter_context(tc.tile_pool(name="xs", bufs=2))
        qs = ctx.enter_context(tc.tile_pool(name="qs", bufs=2))
        rp = ctx.enter_context(tc.tile_pool(name="rp", bufs=4))
        ep = ctx.enter_context(tc.tile_pool(name="ep", bufs=5))
        ot = ctx.enter_context(tc.tile_pool(name="ot", bufs=2))
        po = ctx.enter_context(tc.tile_pool(name="po", bufs=6))
        nrm = ctx.enter_context(tc.tile_pool(name="nrm", bufs=3))
        rcbp = ctx.enter_context(tc.tile_pool(name="rcbp", bufs=3))
        scps = ctx.enter_context(tc.tile_pool(name="scps", bufs=2, space="PSUM"))
        prps = ctx.enter_context(tc.tile_pool(name="prps", bufs=2, space="PSUM"))
        pvps = ctx.enter_context(tc.tile_pool(name="pvps", bufs=2, space="PSUM"))
        wops = ctx.enter_context(tc.tile_pool(name="wops", bufs=2, space="PSUM"))

        # persistent SBUF tensors
        wq_sb = wpool.tile([128, NDT, EG], BF16)
        wk_sb = wpool.tile([128, NDT, EG], BF16)
        wv_sb = wpool.tile([128, NDT, EG], BF16)
        wo_sb = wpool.tile([128, HPG, D], BF16)
        cs_sb = wpool.tile([128, S], BF16)
        sn_sb = wpool.tile([128, S], BF16)
        mk_sb = wpool.tile([128, 4, STRIP], BF16)
        jt_sb = wpool.tile([HD, HD], BF16)
        KT_sb = kv.tile([128, HPG, S], BF16)       # [e, h, sk] rope'd K^T
        V_sb = kv.tile([128, S // 128, EG], BF16)  # [sk, sk_tile, e]

        xt0 = xs.tile([128, NDT, STRIP], BF16, tag="xt")
        x0r = xT_d[:, 0:STRIP].rearrange("(t p) s -> p t s", p=128)

        # --- DMA issue order tuned for the critical path: the first q
        # projection needs wq chunk 0 + the first x d-tiles.
        wqr = wqT_d.rearrange("(t p) e -> p t e", p=128)
        wkr = wkT_d.rearrange("(t p) e -> p t e", p=128)
        wvr = wvT_d.rearrange("(t p) e -> p t e", p=128)
        nc.sync.dma_start(out=wq_sb[:, 0:1, :], in_=wqr[:, 0:1, :])
        nc.sync.dma_start(out=xt0[:, 0:1, :], in_=x0r[:, 0:1, :])
        nc.sync.dma_start(out=wq_sb[:, 1:2, :], in_=wqr[:, 1:2, :])
        nc.sync.dma_start(out=xt0[:, 1:2, :], in_=x0r[:, 1:2, :])
        nc.sync.dma_start(out=wq_sb[:, 2:4, :], in_=wqr[:, 2:4, :])
        nc.sync.dma_start(out=xt0[:, 2:4, :], in_=x0r[:, 2:4, :])
        for c0 in range(4, NDT, 4):
            nc.sync.dma_start(out=wq_sb[:, c0:c0 + 4, :], in_=wqr[:, c0:c0 + 4, :])
            nc.sync.dma_start(out=xt0[:, c0:c0 + 4, :], in_=x0r[:, c0:c0 + 4, :])
        for c0 in range(0, NDT, 4):
            nc.sync.dma_start(out=wk_sb[:, c0:c0 + 4, :], in_=wkr[:, c0:c0 + 4, :])
        nc.sync.dma_start(out=cs_sb, in_=cs_d)
        nc.sync.dma_start(out=sn_sb, in_=sn_d)
        nc.sync.dma_start(out=jt_sb, in_=jt_d)
        xt1 = xs.tile([128, NDT, STRIP], BF16, tag="xt")
        nc.sync.dma_start(
            out=xt1,
            in_=xT_d[:, STRIP:2 * STRIP].rearrange("(t p) s -> p t s", p=128))
        nc.sync.dma_start(out=mk_sb, in_=mk_d)
        for c0 in range(0, NDT, 4):
            nc.sync.dma_start(out=wv_sb[:, c0:c0 + 4, :], in_=wvr[:, c0:c0 + 4, :])
        nc.sync.dma_start(out=wo_sb, in_=woT_d.rearrange("(t p) n -> p t n", p=128))

        for j in range(NSTRIP):
            s0 = j * STRIP
            if j == 0:
                xt = xt0
            elif j == 1:
                xt = xt1
            else:
                xt = xs.tile([128, NDT, STRIP], BF16, tag="xt")
                nc.sync.dma_start(
                    out=xt,
                    in_=xT_d[:, s0:s0 + STRIP].rearrange("(t p) s -> p t s", p=128))
            qt = qs.tile([128, HPG, STRIP], BF16, tag="qt")

            # --- projections + RoPE for this strip ---
            if j == 0:
                # strip 0: chunk-outer over the 4-dt DMA chunks so the first
                # matmuls start as soon as (wq chunk 0, x chunk 0) land; the
                # 4 concurrent head accumulators borrow the idle sc psum bufs
                qp0 = prps.tile([128, STRIP], F32, tag="mm")
                qp1 = prps.tile([128, STRIP], F32, tag="mm")
                qp2 = scps.tile([128, STRIP], F32, tag="sc")
                qp3 = scps.tile([128, STRIP], F32, tag="sc")
                qps0 = [qp0, qp1, qp2, qp3]
                for c0, cw in ((0, 1), (1, 1), (2, 2), (4, 4), (8, 4), (12, 4)):
                    for h in range(HPG):
                        e0 = h * HD
                        for dt in range(c0, c0 + cw):
                            nc.tensor.matmul(qps0[h],
                                             lhsT=wq_sb[:, dt, e0:e0 + HD],
                                             rhs=xt[:, dt, :],
                                             start=(dt == 0),
                                             stop=(dt == NDT - 1))
            for h in range(HPG):
                e0 = h * HD
                if j == 0:
                    q_ps = qps0[h]
                else:
                    q_ps = prps.tile([128, STRIP], F32, tag="mm")
                    for dt in range(NDT):
                        nc.tensor.matmul(q_ps, lhsT=wq_sb[:, dt, e0:e0 + HD],
                                         rhs=xt[:, dt, :],
                                         start=(dt == 0), stop=(dt == NDT - 1))
                q_sb = rp.tile([128, STRIP], BF16, tag="qsb")
                nc.scalar.copy(q_sb, q_ps)
                if j == 0:
                    jq_ps = pvps.tile([128, STRIP], F32, tag="pv")
                else:
                    jq_ps = prps.tile([128, STRIP], F32, tag="mm")
                nc.tensor.matmul(jq_ps, lhsT=jt_sb, rhs=q_sb,
                                 start=True, stop=True)
                jq_sb = rp.tile([128, STRIP], BF16, tag="jsb")
                nc.scalar.copy(jq_sb, jq_ps)
                t1 = rp.tile([128, STRIP], BF16, tag="ra")
                nc.vector.tensor_mul(t1, q_sb, cs_sb[:, s0:s0 + STRIP])
                t2 = rp.tile([128, STRIP], BF16, tag="rb")
                nc.vector.tensor_mul(t2, jq_sb, sn_sb[:, s0:s0 + STRIP])
                nc.vector.tensor_add(qt[:, h, :], t1, t2)

                k_ps = prps.tile([128, STRIP], F32, tag="mm")
                for dt in range(NDT):
                    nc.tensor.matmul(k_ps, lhsT=wk_sb[:, dt, e0:e0 + HD],
                                     rhs=xt[:, dt, :],
                                     start=(dt == 0), stop=(dt == NDT - 1))
                k_sb = rp.tile([128, STRIP], BF16, tag="qsb")
                nc.scalar.copy(k_sb, k_ps)
                jk_ps = prps.tile([128, STRIP], F32, tag="mm")
                nc.tensor.matmul(jk_ps, lhsT=jt_sb, rhs=k_sb,
                                 start=True, stop=True)
                jk_sb = rp.tile([128, STRIP], BF16, tag="jsb")
                nc.scalar.copy(jk_sb, jk_ps)
                t3 = rp.tile([128, STRIP], BF16, tag="ra")
                nc.vector.tensor_mul(t3, k_sb, cs_sb[:, s0:s0 + STRIP])
                t4 = rp.tile([128, STRIP], BF16, tag="rb")
                nc.vector.tensor_mul(t4, jk_sb, sn_sb[:, s0:s0 + STRIP])
                nc.vector.tensor_add(KT_sb[:, h, s0:s0 + STRIP], t3, t4)

            for st in range(4):
                v_ps = prps.tile([128, EG], F32, tag="mm")
                for dt in range(NDT):
                    nc.tensor.matmul(v_ps, lhsT=xt[:, dt, st * 128:(st + 1) * 128],
                                     rhs=wv_sb[:, dt, :],
                                     start=(dt == 0), stop=(dt == NDT - 1))
                nc.scalar.copy(V_sb[:, j * 4 + st, :], v_ps)

            # --- attention for this strip ---
            otile = ot.tile([128, HPG, STRIP], BF16, tag="ot")
            nsk = 4 * j + 4
            for h in range(HPG):
                e0 = h * HD
                pv_ps = pvps.tile([128, STRIP], F32, tag="pv")
                acc = None   # bf16 running elementwise sum of masked exps;
                #              the 128-partition reduction happens in f32 on
                #              Pool, so bf16 chain rounding averages out
                for skt in range(nsk):
                    d = skt - 4 * j   # >= 0 on the diagonal block
                    w = STRIP - 128 * d if d > 0 else STRIP
                    sc_ps = scps.tile([128, STRIP], F32, tag="sc")
                    nc.tensor.matmul(sc_ps[:, 0:w],
                                     lhsT=KT_sb[:, h, skt * 128:(skt + 1) * 128],
                                     rhs=qt[:, h, STRIP - w:STRIP],
                                     start=True, stop=True)
                    ex = ep.tile([128, STRIP], BF16, tag="ex")
                    nc.scalar.activation(ex[:, 0:w], sc_ps[:, 0:w], EXP,
                                         scale=SCALE)
                    if d >= 0:
                        exm = ep.tile([128, STRIP], BF16, tag="exm")
                        nc.vector.tensor_mul(exm[:, 0:w], ex[:, 0:w],
                                             mk_sb[:, d, STRIP - w:STRIP])
                    else:
                        exm = ex
                    nc.tensor.matmul(pv_ps[:, STRIP - w:STRIP],
                                     lhsT=V_sb[:, skt, e0:e0 + HD],
                                     rhs=exm[:, 0:w], start=(skt == 0),
                                     stop=(skt == nsk - 1))
                    if skt == 0:
                        acc = exm      # full width; later adds are in-place
                    else:
                        nc.vector.tensor_add(acc[:, STRIP - w:STRIP],
                                             acc[:, STRIP - w:STRIP],
                                             exm[:, 0:w])
                den = nrm.tile([128, STRIP], F32, tag="den")
                nc.gpsimd.partition_all_reduce(den, acc, 128, ReduceOp.add)
                rcb = rcbp.tile([128, STRIP], F32, tag="rcb")
                nc.vector.reciprocal(rcb, den)
                nc.vector.tensor_mul(otile[:, h, :], pv_ps, rcb)

            # --- partial output projection for this strip ---
            for nt in range(NDT):
                if j == NSTRIP - 1 and nt % 4 == 3:
                    pr = scps.tile([128, STRIP], F32, tag="sc")
                elif j == NSTRIP - 1 and nt % 4 == 1:
                    pr = prps.tile([128, STRIP], F32, tag="mm")
                else:
                    pr = wops.tile([128, STRIP], F32, tag="pr")
                for h in range(HPG):
                    nc.tensor.matmul(pr, lhsT=wo_sb[:, h, nt * 128:(nt + 1) * 128],
                                     rhs=otile[:, h, :],
                                     start=(h == 0), stop=(h == HPG - 1))
                pr_sb = po.tile([128, STRIP], BF16, tag="po")
                if nt % 2 == 1:
                    nc.scalar.copy(pr_sb, pr)
                else:
                    nc.vector.tensor_copy(pr_sb, pr)
                nc.sync.dma_start(
                    out=outT_d[nt * 128:(nt + 1) * 128, s0:s0 + STRIP], in_=pr_sb)

    return nc


_PERM = np.concatenate([np.arange(0, HD, 2), np.arange(1, HD, 2)])


def _host_prep(x, wq, wk, wv, wo, freqs_cos, freqs_sin, mask):
    bf16 = ml_dtypes.bfloat16
    x = np.asarray(x, np.float32)
    wq = np.asarray(wq, np.float32)
    wk = np.asarray(wk, np.float32)
    wv = np.asarray(wv, np.float32)
    wo = np.asarray(wo, np.float32)
    cos = np.asarray(freqs_cos, np.float32)   # [S, HD/2]
    sin = np.asarray(freqs_sin, np.float32)
    mask = np.asarray(mask, np.float32)

    cosH = cos.T                               # [64, S]
    sinH = sin.T
    cs = np.vstack([cosH, cosH]).astype(bf16)  # [128, S]
    sn = np.vstack([sinH, sinH]).astype(bf16)

    # multiplicative binary causal mask for the 4 diagonal-tile flavours:
    # mk[k, d, q] = 1 where allowed, 0 where masked
    mk = np.empty((SKT, 4, STRIP), np.float32)
    for d_ in range(4):
        sub = mask[0:STRIP, d_ * SKT:(d_ + 1) * SKT]   # [q, k]
        mk[:, d_, :] = np.where(np.isfinite(sub), 1.0, 0.0).T
    mk = mk.astype(bf16)

    perm_g = np.concatenate([h * HD + _PERM for h in range(HPG)])

    # lhsT of the rope pair-mix matmul: (J q) rows 0:64 = -q[64:128],
    # rows 64:128 = +q[0:64]; jt = J.T
    jt = np.zeros((HD, HD), np.float32)
    jt[np.arange(64), np.arange(64) + 64] = 1.0
    jt[np.arange(64) + 64, np.arange(64)] = -1.0
    jt = jt.astype(bf16)

    in_maps = []
    for c in range(NCORES):
        b, g = c // HPG, c % HPG
        rows = slice(g * EG, (g + 1) * EG)
        wq_g = wq[rows][perm_g]                # [EG, D], head dims permuted
        wk_g = wk[rows][perm_g]
        wv_g = wv[rows]
        in_maps.append({
            "xT": np.ascontiguousarray(x[b].T).astype(bf16),
            "wqT": np.ascontiguousarray(wq_g.T).astype(bf16),
            "wkT": np.ascontiguousarray(wk_g.T).astype(bf16),
            "wvT": np.ascontiguousarray(wv_g.T).astype(bf16),
            "woT": np.ascontiguousarray(wo[:, rows].T).astype(bf16),
            "cs": cs, "sn": sn, "mk": mk, "jt": jt,
        })
    return in_maps


def kernel(x, wq, wk, wv, wo, freqs_cos, freqs_sin, mask, start_pos):
    global LAST_EXEC_NS, LAST_RESULTS
    in_maps = _host_prep(x, wq, wk, wv, wo, freqs_cos, freqs_sin, mask)
    nc = _build_program()
    nc.finalize()
    res = run_bass_kernel_spmd(nc, in_maps, core_ids=list(range(NCORES)),
                               trace=False)
    LAST_EXEC_NS = res.exec_time_ns
    LAST_RESULTS = res
    out = np.empty((B, S, D), np.float32)
    for b in range(B):
        acc = np.zeros((D, S), np.float32)
        for g in range(HPG):
            acc += res.results[b * HPG + g]["outT"].astype(np.float32)
        out[b] = acc.T
    return out


# revision 54
# speedup vs baseline: 1.5156x; 1.0015x over previous
"""Self-contained Trainium2 Bass kernel for multi-head causal attention with RoPE.

Problem: B=2, S=2048, D=2048, H=16 heads (HD=128), fp32 reference:
    q = rope(x @ wq.T), k = rope(x @ wk.T), v = x @ wv.T
    out = softmax(q k^T / sqrt(HD) + causal_mask) @ v @ wo.T

Sharding (8 cores): core c = (b, g) with b = c // 4 (batch), g = c % 4
(head-group of 4 heads).  Each core computes its head-group's attention for
its batch and a partial output projection (columns 512g:512g+512 of the
attention output times the matching wo rows).  The host sums the 4 partial
[D, S] tensors per batch and transposes back to [S, D].

On-chip layout is "transposed": Q^T/K^T are kept as [head_dim, seq] so the
QK^T matmul needs no transposes, scores come out as scoresT[k, q], and
probsT feeds the PV matmul directly (lhsT = V[sk, e]).  RoPE's even/odd pair
mixing becomes a half-partition mix via a PE matmul against a signed
permutation J (head dims of wq/wk are permuted host-side, which cancels in
q.k).  Causal masking: strictly-above-diagonal 128x512 score tiles are
skipped, diagonal tiles are computed at partial width (only columns at or
right of the diagonal) and get a multiplicative binary bf16 mask applied
after exp.  Softmax denominators: masked exp tiles are summed elementwise on
DVE (bf16, in place); the 128-partition reduction runs in f32 on GpSimd
(partition_all_reduce from the attn ucode library), followed by a DVE
reciprocal and the normalization multiply.
"""

import math

import numpy as np
import ml_dtypes

import concourse.bass as bass
import concourse.bacc as bacc
import concourse.mybir as mybir
from concourse.tile import TileContext
from concourse.bass_utils import run_bass_kernel_spmd
from concourse import library_config
from concourse.bass_isa import ReduceOp
from contextlib import ExitStack

B, S, D, H = 2, 2048, 2048, 16
HD = 128          # head dim
HPG = 4           # heads per core (group)
EG = HPG * HD     # 512 head dims per core
NCORES = 8
NSTRIP = 4        # q strips per sequence
STRIP = S // NSTRIP   # 512
SKT = 128         # k tile (partition dim of scoresT)
NDT = D // 128    # 16 contraction tiles for projections
SCALE = 1.0 / math.sqrt(HD)

BF16 = mybir.dt.bfloat16
F32 = mybir.dt.float32

LAST_EXEC_NS = None
LAST_RESULTS = None


def _build_program():
    nc = bacc.Bacc("TRN2", target_bir_lowering=False, debug=False,
                   num_devices=NCORES)
    xT_d = nc.dram_tensor("xT", [D, S], BF16, kind="ExternalInput").ap()
    wqT_d = nc.dram_tensor("wqT", [D, EG], BF16, kind="ExternalInput").ap()
    wkT_d = nc.dram_tensor("wkT", [D, EG], BF16, kind="ExternalInput").ap()
    wvT_d = nc.dram_tensor("wvT", [D, EG], BF16, kind="ExternalInput").ap()
    woT_d = nc.dram_tensor("woT", [EG, D], BF16, kind="ExternalInput").ap()
    cs_d = nc.dram_tensor("cs", [HD, S], BF16, kind="ExternalInput").ap()
    sn_d = nc.dram_tensor("sn", [HD, S], BF16, kind="ExternalInput").ap()
    mk_d = nc.dram_tensor("mk", [SKT, 4, STRIP], BF16, kind="ExternalInput").ap()
    jt_d = nc.dram_tensor("jt", [HD, HD], BF16, kind="ExternalInput").ap()
    outT_d = nc.dram_tensor("outT", [D, S], BF16, kind="ExternalOutput").ap()

    EXP = mybir.ActivationFunctionType.Exp

    with TileContext(nc) as tc, ExitStack() as ctx:
        nc.gpsimd.load_library(library_config.attn)

        wpool = ctx.enter_context(tc.tile_pool(name="wpool", bufs=1))
        kv = ctx.enter_context(tc.tile_pool(name="kv", bufs=1))
        xs = ctx.enter_context(tc.tile_pool(name="xs", bufs=2))
        qs = ctx.enter_context(tc.tile_pool(name="qs", bufs=2))
        rp = ctx.enter_context(tc.tile_pool(name="rp", bufs=4))
        ep = ctx.enter_context(tc.tile_pool(name="ep", bufs=5))
        ot = ctx.enter_context(tc.tile_pool(name="ot", bufs=2))
        po = ctx.enter_context(tc.tile_pool(name="po", bufs=6))
        nrm = ctx.enter_context(tc.tile_pool(name="nrm", bufs=3))
        rcbp = ctx.enter_context(tc.tile_pool(name="rcbp", bufs=3))
        scps = ctx.enter_context(tc.tile_pool(name="scps", bufs=2, space="PSUM"))
        prps = ctx.enter_context(tc.tile_pool(name="prps", bufs=2, space="PSUM"))
        pvps = ctx.enter_context(tc.tile_pool(name="pvps", bufs=2, space="PSUM"))
        wops = ctx.enter_context(tc.tile_pool(name="wops", bufs=2, space="PSUM"))

        # persistent SBUF tensors
        wq_sb = wpool.tile([128, NDT, EG], BF16)
        wk_sb = wpool.tile([128, NDT, EG], BF16)
        wv_sb = wpool.tile([128, NDT, EG], BF16)
        wo_sb = wpool.tile([128, HPG, D], BF16)
        cs_sb = wpool.tile([128, S], BF16)
        sn_sb = wpool.tile([128, S], BF16)
        mk_sb = wpool.tile([128, 4, STRIP], BF16)
        jt_sb = wpool.tile([HD, HD], BF16)
        KT_sb = kv.tile([128, HPG, S], BF16)       # [e, h, sk] rope'd K^T
        V_sb = kv.tile([128, S // 128, EG], BF16)  # [sk, sk_tile, e]

        xt0 = xs.tile([128, NDT, STRIP], BF16, tag="xt")
        x0r = xT_d[:, 0:STRIP].rearrange("(t p) s -> p t s", p=128)

        # --- DMA issue order tuned for the critical path: the first q
        # projection needs wq chunk 0 + the first x d-tiles.
        wqr = wqT_d.rearrange("(t p) e -> p t e", p=128)
        wkr = wkT_d.rearrange("(t p) e -> p t e", p=128)
        wvr = wvT_d.rearrange("(t p) e -> p t e", p=128)
        nc.sync.dma_start(out=wq_sb[:, 0:1, :], in_=wqr[:, 0:1, :])
        nc.sync.dma_start(out=xt0[:, 0:1, :], in_=x0r[:, 0:1, :])
        nc.sync.dma_start(out=wq_sb[:, 1:2, :], in_=wqr[:, 1:2, :])
        nc.sync.dma_start(out=xt0[:, 1:2, :], in_=x0r[:, 1:2, :])
        nc.sync.dma_start(out=wq_sb[:, 2:4, :], in_=wqr[:, 2:4, :])
        nc.sync.dma_start(out=xt0[:, 2:4, :], in_=x0r[:, 2:4, :])
        for c0 in range(4, NDT, 4):
            nc.sync.dma_start(out=wq_sb[:, c0:c0 + 4, :], in_=wqr[:, c0:c0 + 4, :])
            nc.sync.dma_start(out=xt0[:, c0:c0 + 4, :], in_=x0r[:, c0:c0 + 4, :])
        for c0 in range(0, NDT, 4):
            nc.sync.dma_start(out=wk_sb[:, c0:c0 + 4, :], in_=wkr[:, c0:c0 + 4, :])
        nc.sync.dma_start(out=cs_sb, in_=cs_d)
        nc.sync.dma_start(out=sn_sb, in_=sn_d)
        nc.sync.dma_start(out=jt_sb, in_=jt_d)
        xt1 = xs.tile([128, NDT, STRIP], BF16, tag="xt")
        nc.sync.dma_start(
            out=xt1,
            in_=xT_d[:, STRIP:2 * STRIP].rearrange("(t p) s -> p t s", p=128))
        nc.sync.dma_start(out=mk_sb, in_=mk_d)
        for c0 in range(0, NDT, 4):
            nc.sync.dma_start(out=wv_sb[:, c0:c0 + 4, :], in_=wvr[:, c0:c0 + 4, :])
        nc.sync.dma_start(out=wo_sb, in_=woT_d.rearrange("(t p) n -> p t n", p=128))

        for j in range(NSTRIP):
            s0 = j * STRIP
            if j == 0:
                xt = xt0
            elif j == 1:
                xt = xt1
            else:
                xt = xs.tile([128, NDT, STRIP], BF16, tag="xt")
                nc.sync.dma_start(
                    out=xt,
                    in_=xT_d[:, s0:s0 + STRIP].rearrange("(t p) s -> p t s", p=128))
            qt = qs.tile([128, HPG, STRIP], BF16, tag="qt")

            # --- projections + RoPE for this strip ---
            if j == 0:
                # strip 0: chunk-outer over the 4-dt DMA chunks so the first
                # matmuls start as soon as (wq chunk 0, x chunk 0) land; the
                # 4 concurrent head accumulators borrow the idle sc psum bufs
                qp0 = prps.tile([128, STRIP], F32, tag="mm")
                qp1 = prps.tile([128, STRIP], F32, tag="mm")
                qp2 = scps.tile([128, STRIP], F32, tag="sc")
                qp3 = scps.tile([128, STRIP], F32, tag="sc")
                qps0 = [qp0, qp1, qp2, qp3]
                for c0, cw in ((0, 1), (1, 1), (2, 2), (4, 4), (8, 4), (12, 4)):
                    for h in range(HPG):
                        e0 = h * HD
                        for dt in range(c0, c0 + cw):
                            nc.tensor.matmul(qps0[h],
                                             lhsT=wq_sb[:, dt, e0:e0 + HD],
                                             rhs=xt[:, dt, :],
                                             start=(dt == 0),
                                             stop=(dt == NDT - 1))
            for h in range(HPG):
                e0 = h * HD
                if j == 0:
                    q_ps = qps0[h]
                else:
                    q_ps = prps.tile([128, STRIP], F32, tag="mm")
                    for dt in range(NDT):
                        nc.tensor.matmul(q_ps, lhsT=wq_sb[:, dt, e0:e0 + HD],
                                         rhs=xt[:, dt, :],
                                         start=(dt == 0), stop=(dt == NDT - 1))
                q_sb = rp.tile([128, STRIP], BF16, tag="qsb")
                nc.scalar.copy(q_sb, q_ps)
                if j == 0:
                    jq_ps = pvps.tile([128, STRIP], F32, tag="pv")
                else:
                    jq_ps = prps.tile([128, STRIP], F32, tag="mm")
                nc.tensor.matmul(jq_ps, lhsT=jt_sb, rhs=q_sb,
                                 start=True, stop=True)
                jq_sb = rp.tile([128, STRIP], BF16, tag="jsb")
                nc.scalar.copy(jq_sb, jq_ps)
                t1 = rp.tile([128, STRIP], BF16, tag="ra")
                nc.vector.tensor_mul(t1, q_sb, cs_sb[:, s0:s0 + STRIP])
                t2 = rp.tile([128, STRIP], BF16, tag="rb")
                nc.vector.tensor_mul(t2, jq_sb, sn_sb[:, s0:s0 + STRIP])
                nc.vector.tensor_add(qt[:, h, :], t1, t2)

                k_ps = prps.tile([128, STRIP], F32, tag="mm")
                for dt in range(NDT):
                    nc.tensor.matmul(k_ps, lhsT=wk_sb[:, dt, e0:e0 + HD],
                                     rhs=xt[:, dt, :],
                                     start=(dt == 0), stop=(dt == NDT - 1))
                k_sb = rp.tile([128, STRIP], BF16, tag="qsb")
                nc.vector.tensor_copy(k_sb, k_ps)
                jk_ps = prps.tile([128, STRIP], F32, tag="mm")
                nc.tensor.matmul(jk_ps, lhsT=jt_sb, rhs=k_sb,
                                 start=True, stop=True)
                jk_sb = rp.tile([128, STRIP], BF16, tag="jsb")
                nc.scalar.copy(jk_sb, jk_ps)
                t3 = rp.tile([128, STRIP], BF16, tag="ra")
                nc.vector.tensor_mul(t3, k_sb, cs_sb[:, s0:s0 + STRIP])
                t4 = rp.tile([128, STRIP], BF16, tag="rb")
                nc.vector.tensor_mul(t4, jk_sb, sn_sb[:, s0:s0 + STRIP])
                nc.vector.tensor_add(KT_sb[:, h, s0:s0 + STRIP], t3, t4)

            for st in range(4):
                v_ps = prps.tile([128, EG], F32, tag="mm")
                for dt in range(NDT):
                    nc.tensor.matmul(v_ps, lhsT=xt[:, dt, st * 128:(st + 1) * 128],
                                     rhs=wv_sb[:, dt, :],
                                     start=(dt == 0), stop=(dt == NDT - 1))
                nc.vector.tensor_copy(V_sb[:, j * 4 + st, :], v_ps)

            # --- attention for this strip ---
            otile = ot.tile([128, HPG, STRIP], BF16, tag="ot")
            nsk = 4 * j + 4
            for h in range(HPG):
                e0 = h * HD
                pv_ps = pvps.tile([128, STRIP], F32, tag="pv")
                acc = None   # bf16 running elementwise sum of masked exps;
                #              the 128-partition reduction happens in f32 on
                #              Pool, so bf16 chain rounding averages out
                for skt in range(nsk):
                    d = skt - 4 * j   # >= 0 on the diagonal block
                    w = STRIP - 128 * d if d > 0 else STRIP
                    sc_ps = scps.tile([128, STRIP], F32, tag="sc")
                    nc.tensor.matmul(sc_ps[:, 0:w],
                                     lhsT=KT_sb[:, h, skt * 128:(skt + 1) * 128],
                                     rhs=qt[:, h, STRIP - w:STRIP],
                                     start=True, stop=True)
                    ex = ep.tile([128, STRIP], BF16, tag="ex")
                    nc.scalar.activation(ex[:, 0:w], sc_ps[:, 0:w], EXP,
                                         scale=SCALE)
                    if d >= 0:
                        exm = ep.tile([128, STRIP], BF16, tag="exm")
                        nc.vector.tensor_mul(exm[:, 0:w], ex[:, 0:w],
                                             mk_sb[:, d, STRIP - w:STRIP])
                    else:
                        exm = ex
                    nc.tensor.matmul(pv_ps[:, STRIP - w:STRIP],
                                     lhsT=V_sb[:, skt, e0:e0 + HD],
                                     rhs=exm[:, 0:w], start=(skt == 0),
                                     stop=(skt == nsk - 1))
                    if skt == 0:
                        acc = exm      # full width; later adds are in-place
                    else:
                        nc.vector.tensor_add(acc[:, STRIP - w:STRIP],
                                             acc[:, STRIP - w:STRIP],
                                             exm[:, 0:w])
                den = nrm.tile([128, STRIP], F32, tag="den")
                nc.gpsimd.partition_all_reduce(den, acc, 128, ReduceOp.add)
                rcb = rcbp.tile([128, STRIP], F32, tag="rcb")
                nc.vector.reciprocal(rcb, den)
                nc.vector.tensor_mul(otile[:, h, :], pv_ps, rcb)

            # --- partial output projection for this strip ---
            for nt in range(NDT):
                if j == NSTRIP - 1 and nt % 4 == 3:
                    pr = scps.tile([128, STRIP], F32, tag="sc")
                elif j == NSTRIP - 1 and nt % 4 == 1:
                    pr = prps.tile([128, STRIP], F32, tag="mm")
                else:
                    pr = wops.tile([128, STRIP], F32, tag="pr")
                for h in range(HPG):
                    nc.tensor.matmul(pr, lhsT=wo_sb[:, h, nt * 128:(nt + 1) * 128],
                                     rhs=otile[:, h, :],
                                     start=(h == 0), stop=(h == HPG - 1))
                pr_sb = po.tile([128, STRIP], BF16, tag="po")
                if nt % 2 == 1:
                    nc.scalar.copy(pr_sb, pr)
                else:
                    nc.vector.tensor_copy(pr_sb, pr)
                nc.sync.dma_start(
                    out=outT_d[nt * 128:(nt + 1) * 128, s0:s0 + STRIP], in_=pr_sb)

    return nc


_PERM = np.concatenate([np.arange(0, HD, 2), np.arange(1, HD, 2)])


def _host_prep(x, wq, wk, wv, wo, freqs_cos, freqs_sin, mask):
    bf16 = ml_dtypes.bfloat16
    x = np.asarray(x, np.float32)
    wq = np.asarray(wq, np.float32)
    wk = np.asarray(wk, np.float32)
    wv = np.asarray(wv, np.float32)
    wo = np.asarray(wo, np.float32)
    cos = np.asarray(freqs_cos, np.float32)   # [S, HD/2]
    sin = np.asarray(freqs_sin, np.float32)
    mask = np.asarray(mask, np.float32)

    cosH = cos.T                               # [64, S]
    sinH = sin.T
    cs = np.vstack([cosH, cosH]).astype(bf16)  # [128, S]
    sn = np.vstack([sinH, sinH]).astype(bf16)

    # multiplicative binary causal mask for the 4 diagonal-tile flavours:
    # mk[k, d, q] = 1 where allowed, 0 where masked
    mk = np.empty((SKT, 4, STRIP), np.float32)
    for d_ in range(4):
        sub = mask[0:STRIP, d_ * SKT:(d_ + 1) * SKT]   # [q, k]
        mk[:, d_, :] = np.where(np.isfinite(sub), 1.0, 0.0).T
    mk = mk.astype(bf16)

    perm_g = np.concatenate([h * HD + _PERM for h in range(HPG)])

    # lhsT of the rope pair-mix matmul: (J q) rows 0:64 = -q[64:128],
    # rows 64:128 = +q[0:64]; jt = J.T
    jt = np.zeros((HD, HD), np.float32)
    jt[np.arange(64), np.arange(64) + 64] = 1.0
    jt[np.arange(64) + 64, np.arange(64)] = -1.0
    jt = jt.astype(bf16)

    in_maps = []
    for c in range(NCORES):
        b, g = c // HPG, c % HPG
        rows = slice(g * EG, (g + 1) * EG)
        wq_g = wq[rows][perm_g]                # [EG, D], head dims permuted
        wk_g = wk[rows][perm_g]
        wv_g = wv[rows]
        in_maps.append({
            "xT": np.ascontiguousarray(x[b].T).astype(bf16),
            "wqT": np.ascontiguousarray(wq_g.T).astype(bf16),
            "wkT": np.ascontiguousarray(wk_g.T).astype(bf16),
            "wvT": np.ascontiguousarray(wv_g.T).astype(bf16),
            "woT": np.ascontiguousarray(wo[:, rows].T).astype(bf16),
            "cs": cs, "sn": sn, "mk": mk, "jt": jt,
        })
    return in_maps


def kernel(x, wq, wk, wv, wo, freqs_cos, freqs_sin, mask, start_pos):
    global LAST_EXEC_NS, LAST_RESULTS
    in_maps = _host_prep(x, wq, wk, wv, wo, freqs_cos, freqs_sin, mask)
    nc = _build_program()
    nc.finalize()
    res = run_bass_kernel_spmd(nc, in_maps, core_ids=list(range(NCORES)),
                               trace=False)
    LAST_EXEC_NS = res.exec_time_ns
    LAST_RESULTS = res
    out = np.empty((B, S, D), np.float32)
    for b in range(B):
        acc = np.zeros((D, S), np.float32)
        for g in range(HPG):
            acc += res.results[b * HPG + g]["outT"].astype(np.float32)
        out[b] = acc.T
    return out


# revision 74
# speedup vs baseline: 1.5660x; 1.0333x over previous
"""Self-contained Trainium2 Bass kernel for multi-head causal attention with RoPE.

Problem: B=2, S=2048, D=2048, H=16 heads (HD=128), fp32 reference:
    q = rope(x @ wq.T), k = rope(x @ wk.T), v = x @ wv.T
    out = softmax(q k^T / sqrt(HD) + causal_mask) @ v @ wo.T

Sharding (8 cores): core c = (b, g) with b = c // 4 (batch), g = c % 4
(head-group of 4 heads).  Each core computes its head-group's attention for
its batch and a partial output projection (columns 512g:512g+512 of the
attention output times the matching wo rows).  The host sums the 4 partial
[D, S] tensors per batch and transposes back to [S, D].

On-chip layout is "transposed": Q^T/K^T are kept as [head_dim, seq] so the
QK^T matmul needs no transposes, scores come out as scoresT[k, q], and
probsT feeds the PV matmul directly (lhsT = V[sk, e]).  RoPE's even/odd pair
mixing becomes a half-partition mix via a PE matmul against a signed
permutation J (head dims of wq/wk are permuted host-side, which cancels in
q.k).  Causal masking: strictly-above-diagonal 128x512 score tiles are
skipped, diagonal tiles are computed at partial width (only columns at or
right of the diagonal) and get a multiplicative binary bf16 mask applied
after exp.  Softmax denominators: masked exp tiles are summed elementwise on
DVE (bf16, in place); the 128-partition reduction runs in f32 on GpSimd
(partition_all_reduce from the attn ucode library), followed by a DVE
reciprocal and the normalization multiply.
"""

import math

import numpy as np
import ml_dtypes

import concourse.bass as bass
import concourse.bacc as bacc
import concourse.mybir as mybir
from concourse.tile import TileContext
from concourse.bass_utils import run_bass_kernel_spmd
from concourse import library_config
from concourse.bass_isa import ReduceOp
from contextlib import ExitStack

B, S, D, H = 2, 2048, 2048, 16
HD = 128          # head dim
HPG = 4           # heads per core (group)
EG = HPG * HD     # 512 head dims per core
NCORES = 8
NSTRIP = 4        # q strips per sequence
STRIP = S // NSTRIP   # 512
SKT = 128         # k tile (partition dim of scoresT)
NDT = D // 128    # 16 contraction tiles for projections
SCALE = 1.0 / math.sqrt(HD)

BF16 = mybir.dt.bfloat16
F32 = mybir.dt.float32

LAST_EXEC_NS = None
LAST_RESULTS = None


def _build_program():
    nc = bacc.Bacc("TRN2", target_bir_lowering=False, debug=False,
                   num_devices=NCORES)
    xT_d = nc.dram_tensor("xT", [D, S], BF16, kind="ExternalInput").ap()
    wqT_d = nc.dram_tensor("wqT", [D, EG], BF16, kind="ExternalInput").ap()
    wkT_d = nc.dram_tensor("wkT", [D, EG], BF16, kind="ExternalInput").ap()
    wvT_d = nc.dram_tensor("wvT", [D, EG], BF16, kind="ExternalInput").ap()
    woT_d = nc.dram_tensor("woT", [EG, D], BF16, kind="ExternalInput").ap()
    cs_d = nc.dram_tensor("cs", [HD, S], BF16, kind="ExternalInput").ap()
    sn_d = nc.dram_tensor("sn", [HD, S], BF16, kind="ExternalInput").ap()
    mk_d = nc.dram_tensor("mk", [SKT, 4, STRIP], BF16, kind="ExternalInput").ap()
    jt_d = nc.dram_tensor("jt", [HD, HD], BF16, kind="ExternalInput").ap()
    outT_d = nc.dram_tensor("outT", [D, S], BF16, kind="ExternalOutput").ap()

    EXP = mybir.ActivationFunctionType.Exp

    with TileContext(nc) as tc, ExitStack() as ctx:
        nc.gpsimd.load_library(library_config.attn)

        wpool = ctx.enter_context(tc.tile_pool(name="wpool", bufs=1))
        kv = ctx.enter_context(tc.tile_pool(name="kv", bufs=1))
        xs = ctx.enter_context(tc.tile_pool(name="xs", bufs=2))
        qs = ctx.enter_context(tc.tile_pool(name="qs", bufs=2))
        rp = ctx.enter_context(tc.tile_pool(name="rp", bufs=4))
        ep = ctx.enter_context(tc.tile_pool(name="ep", bufs=5))
        ot = ctx.enter_context(tc.tile_pool(name="ot", bufs=3))
        po = ctx.enter_context(tc.tile_pool(name="po", bufs=6))
        nrm = ctx.enter_context(tc.tile_pool(name="nrm", bufs=3))
        rcbp = ctx.enter_context(tc.tile_pool(name="rcbp", bufs=3))
        scps = ctx.enter_context(tc.tile_pool(name="scps", bufs=2, space="PSUM"))
        prps = ctx.enter_context(tc.tile_pool(name="prps", bufs=2, space="PSUM"))
        pvps = ctx.enter_context(tc.tile_pool(name="pvps", bufs=2, space="PSUM"))
        wops = ctx.enter_context(tc.tile_pool(name="wops", bufs=2, space="PSUM"))

        # persistent SBUF tensors
        wq_sb = wpool.tile([128, NDT, EG], BF16)
        wk_sb = wpool.tile([128, NDT, EG], BF16)
        wv_sb = wpool.tile([128, NDT, EG], BF16)
        wo_sb = wpool.tile([128, HPG, D], BF16)
        cs_sb = wpool.tile([128, S], BF16)
        sn_sb = wpool.tile([128, S], BF16)
        mk_sb = wpool.tile([128, 4, STRIP], BF16)
        jt_sb = wpool.tile([HD, HD], BF16)
        KT_sb = kv.tile([128, HPG, S], BF16)       # [e, h, sk] rope'd K^T
        V_sb = kv.tile([128, S // 128, EG], BF16)  # [sk, sk_tile, e]

        xt0 = xs.tile([128, NDT, STRIP], BF16, tag="xt")
        x0r = xT_d[:, 0:STRIP].rearrange("(t p) s -> p t s", p=128)

        # --- DMA issue order tuned for the critical path: the first q
        # projection needs wq chunk 0 + the first x d-tiles.
        wqr = wqT_d.rearrange("(t p) e -> p t e", p=128)
        wkr = wkT_d.rearrange("(t p) e -> p t e", p=128)
        wvr = wvT_d.rearrange("(t p) e -> p t e", p=128)
        nc.sync.dma_start(out=wq_sb[:, 0:1, :], in_=wqr[:, 0:1, :])
        nc.sync.dma_start(out=xt0[:, 0:1, :], in_=x0r[:, 0:1, :])
        nc.sync.dma_start(out=wq_sb[:, 1:2, :], in_=wqr[:, 1:2, :])
        nc.sync.dma_start(out=xt0[:, 1:2, :], in_=x0r[:, 1:2, :])
        nc.sync.dma_start(out=wq_sb[:, 2:4, :], in_=wqr[:, 2:4, :])
        nc.sync.dma_start(out=xt0[:, 2:4, :], in_=x0r[:, 2:4, :])
        nc.sync.dma_start(out=wq_sb[:, 4:6, :], in_=wqr[:, 4:6, :])
        nc.sync.dma_start(out=xt0[:, 4:6, :], in_=x0r[:, 4:6, :])
        nc.sync.dma_start(out=wq_sb[:, 6:8, :], in_=wqr[:, 6:8, :])
        nc.sync.dma_start(out=xt0[:, 6:8, :], in_=x0r[:, 6:8, :])
        for c0 in range(8, NDT, 4):
            nc.sync.dma_start(out=wq_sb[:, c0:c0 + 4, :], in_=wqr[:, c0:c0 + 4, :])
            nc.sync.dma_start(out=xt0[:, c0:c0 + 4, :], in_=x0r[:, c0:c0 + 4, :])
        for c0 in range(0, NDT, 4):
            nc.sync.dma_start(out=wk_sb[:, c0:c0 + 4, :], in_=wkr[:, c0:c0 + 4, :])
        nc.sync.dma_start(out=jt_sb, in_=jt_d)
        nc.sync.dma_start(out=wv_sb[:, 0:4, :], in_=wvr[:, 0:4, :])
        nc.sync.dma_start(out=cs_sb, in_=cs_d)
        nc.sync.dma_start(out=sn_sb, in_=sn_d)
        for c0 in range(4, NDT, 4):
            nc.sync.dma_start(out=wv_sb[:, c0:c0 + 4, :], in_=wvr[:, c0:c0 + 4, :])
        nc.sync.dma_start(out=mk_sb, in_=mk_d)
        xt1 = xs.tile([128, NDT, STRIP], BF16, tag="xt")
        nc.sync.dma_start(
            out=xt1,
            in_=xT_d[:, STRIP:2 * STRIP].rearrange("(t p) s -> p t s", p=128))
        nc.sync.dma_start(out=wo_sb, in_=woT_d.rearrange("(t p) n -> p t n", p=128))

        deferred = []

        def _emit_wo(j, otile):
            s0 = j * STRIP
            for nt in range(NDT):
                if j == NSTRIP - 1 and nt % 4 == 3:
                    pr = scps.tile([128, STRIP], F32, tag="sc")
                elif j == NSTRIP - 1 and nt % 4 == 1:
                    pr = prps.tile([128, STRIP], F32, tag="mm")
                else:
                    pr = wops.tile([128, STRIP], F32, tag="pr")
                for h in range(HPG):
                    nc.tensor.matmul(pr, lhsT=wo_sb[:, h, nt * 128:(nt + 1) * 128],
                                     rhs=otile[:, h, :],
                                     start=(h == 0), stop=(h == HPG - 1))
                pr_sb = po.tile([128, STRIP], BF16, tag="po")
                if nt % 2 == 1:
                    nc.scalar.copy(pr_sb, pr)
                else:
                    nc.vector.tensor_copy(pr_sb, pr)
                nc.sync.dma_start(
                    out=outT_d[nt * 128:(nt + 1) * 128, s0:s0 + STRIP], in_=pr_sb)

        for j in range(NSTRIP):
            s0 = j * STRIP
            if j == 0:
                xt = xt0
            elif j == 1:
                xt = xt1
            else:
                xt = xs.tile([128, NDT, STRIP], BF16, tag="xt")
                nc.sync.dma_start(
                    out=xt,
                    in_=xT_d[:, s0:s0 + STRIP].rearrange("(t p) s -> p t s", p=128))
            qt = qs.tile([128, HPG, STRIP], BF16, tag="qt")

            # --- projections + RoPE for this strip ---
            if j == 0:
                # strip 0: chunk-outer over the 4-dt DMA chunks so the first
                # matmuls start as soon as (wq chunk 0, x chunk 0) land; the
                # 4 concurrent head accumulators borrow the idle sc psum bufs
                qp0 = prps.tile([128, STRIP], F32, tag="mm")
                qp1 = prps.tile([128, STRIP], F32, tag="mm")
                qp2 = scps.tile([128, STRIP], F32, tag="sc")
                qp3 = scps.tile([128, STRIP], F32, tag="sc")
                qps0 = [qp0, qp1, qp2, qp3]
                for c0, cw in ((0, 1), (1, 1), (2, 2), (4, 2), (6, 2), (8, 4), (12, 4)):
                    for h in range(HPG):
                        e0 = h * HD
                        for dt in range(c0, c0 + cw):
                            nc.tensor.matmul(qps0[h],
                                             lhsT=wq_sb[:, dt, e0:e0 + HD],
                                             rhs=xt[:, dt, :],
                                             start=(dt == 0),
                                             stop=(dt == NDT - 1))
            for h in range(HPG):
                e0 = h * HD
                if j == 0:
                    q_ps = qps0[h]
                else:
                    q_ps = prps.tile([128, STRIP], F32, tag="mm")
                    for dt in range(NDT):
                        nc.tensor.matmul(q_ps, lhsT=wq_sb[:, dt, e0:e0 + HD],
                                         rhs=xt[:, dt, :],
                                         start=(dt == 0), stop=(dt == NDT - 1))
                q_sb = rp.tile([128, STRIP], BF16, tag="qsb")
                nc.scalar.copy(q_sb, q_ps)
                if j == 0:
                    jq_ps = pvps.tile([128, STRIP], F32, tag="pv")
                else:
                    jq_ps = prps.tile([128, STRIP], F32, tag="mm")
                nc.tensor.matmul(jq_ps, lhsT=jt_sb, rhs=q_sb,
                                 start=True, stop=True)
                jq_sb = rp.tile([128, STRIP], BF16, tag="jsb")
                nc.scalar.copy(jq_sb, jq_ps)
                t1 = rp.tile([128, STRIP], BF16, tag="ra")
                nc.vector.tensor_mul(t1, q_sb, cs_sb[:, s0:s0 + STRIP])
                t2 = rp.tile([128, STRIP], BF16, tag="rb")
                nc.vector.tensor_mul(t2, jq_sb, sn_sb[:, s0:s0 + STRIP])
                nc.vector.tensor_add(qt[:, h, :], t1, t2)

                k_ps = prps.tile([128, STRIP], F32, tag="mm")
                for dt in range(NDT):
                    nc.tensor.matmul(k_ps, lhsT=wk_sb[:, dt, e0:e0 + HD],
                                     rhs=xt[:, dt, :],
                                     start=(dt == 0), stop=(dt == NDT - 1))
                k_sb = rp.tile([128, STRIP], BF16, tag="qsb")
                nc.vector.tensor_copy(k_sb, k_ps)
                jk_ps = prps.tile([128, STRIP], F32, tag="mm")
                nc.tensor.matmul(jk_ps, lhsT=jt_sb, rhs=k_sb,
                                 start=True, stop=True)
                jk_sb = rp.tile([128, STRIP], BF16, tag="jsb")
                nc.scalar.copy(jk_sb, jk_ps)
                t3 = rp.tile([128, STRIP], BF16, tag="ra")
                nc.vector.tensor_mul(t3, k_sb, cs_sb[:, s0:s0 + STRIP])
                t4 = rp.tile([128, STRIP], BF16, tag="rb")
                nc.vector.tensor_mul(t4, jk_sb, sn_sb[:, s0:s0 + STRIP])
                nc.vector.tensor_add(KT_sb[:, h, s0:s0 + STRIP], t3, t4)

            for st in range(4):
                v_ps = prps.tile([128, EG], F32, tag="mm")
                for dt in range(NDT):
                    nc.tensor.matmul(v_ps, lhsT=xt[:, dt, st * 128:(st + 1) * 128],
                                     rhs=wv_sb[:, dt, :],
                                     start=(dt == 0), stop=(dt == NDT - 1))
                nc.vector.tensor_copy(V_sb[:, j * 4 + st, :], v_ps)

            # --- attention for this strip ---
            otile = ot.tile([128, HPG, STRIP], BF16, tag="ot")
            nsk = 4 * j + 4
            for h in range(HPG):
                if j == NSTRIP - 1 and deferred:
                    _emit_wo(*deferred.pop(0))
                e0 = h * HD
                pv_ps = pvps.tile([128, STRIP], F32, tag="pv")
                acc = None   # bf16 running elementwise sum of masked exps;
                #              the 128-partition reduction happens in f32 on
                #              Pool, so bf16 chain rounding averages out
                for skt in range(nsk):
                    d = skt - 4 * j   # >= 0 on the diagonal block
                    w = STRIP - 128 * d if d > 0 else STRIP
                    sc_ps = scps.tile([128, STRIP], F32, tag="sc")
                    nc.tensor.matmul(sc_ps[:, 0:w],
                                     lhsT=KT_sb[:, h, skt * 128:(skt + 1) * 128],
                                     rhs=qt[:, h, STRIP - w:STRIP],
                                     start=True, stop=True)
                    ex = ep.tile([128, STRIP], BF16, tag="ex")
                    nc.scalar.activation(ex[:, 0:w], sc_ps[:, 0:w], EXP,
                                         scale=SCALE)
                    if d >= 0:
                        exm = ep.tile([128, STRIP], BF16, tag="exm")
                        nc.vector.tensor_mul(exm[:, 0:w], ex[:, 0:w],
                                             mk_sb[:, d, STRIP - w:STRIP])
                    else:
                        exm = ex
                    nc.tensor.matmul(pv_ps[:, STRIP - w:STRIP],
                                     lhsT=V_sb[:, skt, e0:e0 + HD],
                                     rhs=exm[:, 0:w], start=(skt == 0),
                                     stop=(skt == nsk - 1))
                    if skt == 0:
                        acc = exm      # full width; later adds are in-place
                    else:
                        nc.vector.tensor_add(acc[:, STRIP - w:STRIP],
                                             acc[:, STRIP - w:STRIP],
                                             exm[:, 0:w])
                if h == HPG - 1:
                    # last head gates the wo stage: split the normalization
                    # chain to shorten the critical path
                    nsp = 4 if j == NSTRIP - 1 else 2
                    den = nrm.tile([128, STRIP], F32, tag="den")
                    rcb = rcbp.tile([128, STRIP], F32, tag="rcb")
                    HW_ = STRIP // nsp
                    for ci in range(nsp):
                        cs_ = slice(ci * HW_, (ci + 1) * HW_)
                        nc.gpsimd.partition_all_reduce(den[:, cs_],
                                                       acc[:, cs_], 128,
                                                       ReduceOp.add)
                        nc.vector.reciprocal(rcb[:, cs_], den[:, cs_])
                        nc.vector.tensor_mul(otile[:, h, cs_],
                                             pv_ps[:, cs_], rcb[:, cs_])
                else:
                    den = nrm.tile([128, STRIP], F32, tag="den")
                    nc.gpsimd.partition_all_reduce(den, acc, 128, ReduceOp.add)
                    rcb = rcbp.tile([128, STRIP], F32, tag="rcb")
                    nc.vector.reciprocal(rcb, den)
                    nc.vector.tensor_mul(otile[:, h, :], pv_ps, rcb)

            # --- partial output projection for this strip ---
            # strip 2's wo is deferred into strip 3's attention window (its
            # own window is already filled by strip-3 projections; strip 3's
            # ACT-bound attention needs the PE work)
            if j < NSTRIP - 1:
                deferred.append((j, otile))
            else:
                _emit_wo(j, otile)

    return nc


_PERM = np.concatenate([np.arange(0, HD, 2), np.arange(1, HD, 2)])


def _host_prep(x, wq, wk, wv, wo, freqs_cos, freqs_sin, mask):
    bf16 = ml_dtypes.bfloat16
    x = np.asarray(x, np.float32)
    wq = np.asarray(wq, np.float32)
    wk = np.asarray(wk, np.float32)
    wv = np.asarray(wv, np.float32)
    wo = np.asarray(wo, np.float32)
    cos = np.asarray(freqs_cos, np.float32)   # [S, HD/2]
    sin = np.asarray(freqs_sin, np.float32)
    mask = np.asarray(mask, np.float32)

    cosH = cos.T                               # [64, S]
    sinH = sin.T
    cs = np.vstack([cosH, cosH]).astype(bf16)  # [128, S]
    sn = np.vstack([sinH, sinH]).astype(bf16)

    # multiplicative binary causal mask for the 4 diagonal-tile flavours:
    # mk[k, d, q] = 1 where allowed, 0 where masked
    mk = np.empty((SKT, 4, STRIP), np.float32)
    for d_ in range(4):
        sub = mask[0:STRIP, d_ * SKT:(d_ + 1) * SKT]   # [q, k]
        mk[:, d_, :] = np.where(np.isfinite(sub), 1.0, 0.0).T
    mk = mk.astype(bf16)

    perm_g = np.concatenate([h * HD + _PERM for h in range(HPG)])

    # lhsT of the rope pair-mix matmul: (J q) rows 0:64 = -q[64:128],
    # rows 64:128 = +q[0:64]; jt = J.T
    jt = np.zeros((HD, HD), np.float32)
    jt[np.arange(64), np.arange(64) + 64] = 1.0
    jt[np.arange(64) + 64, np.arange(64)] = -1.0
    jt = jt.astype(bf16)

    in_maps = []
    for c in range(NCORES):
        b, g = c // HPG, c % HPG
        rows = slice(g * EG, (g + 1) * EG)
        wq_g = wq[rows][perm_g]                # [EG, D], head dims permuted
        wk_g = wk[rows][perm_g]
        wv_g = wv[rows]
        in_maps.append({
            "xT": np.ascontiguousarray(x[b].T).astype(bf16),
            "wqT": np.ascontiguousarray(wq_g.T).astype(bf16),
            "wkT": np.ascontiguousarray(wk_g.T).astype(bf16),
            "wvT": np.ascontiguousarray(wv_g.T).astype(bf16),
            "woT": np.ascontiguousarray(wo[:, rows].T).astype(bf16),
            "cs": cs, "sn": sn, "mk": mk, "jt": jt,
        })
    return in_maps


def kernel(x, wq, wk, wv, wo, freqs_cos, freqs_sin, mask, start_pos):
    global LAST_EXEC_NS, LAST_RESULTS
    in_maps = _host_prep(x, wq, wk, wv, wo, freqs_cos, freqs_sin, mask)
    nc = _build_program()
    nc.finalize()
    res = run_bass_kernel_spmd(nc, in_maps, core_ids=list(range(NCORES)),
                               trace=False)
    LAST_EXEC_NS = res.exec_time_ns
    LAST_RESULTS = res
    out = np.empty((B, S, D), np.float32)
    for b in range(B):
        acc = np.zeros((D, S), np.float32)
        for g in range(HPG):
            acc += res.results[b * HPG + g]["outT"].astype(np.float32)
        out[b] = acc.T
    return out
